# revision 1
# baseline (speedup 1.0000x reference)
"""CGCNN message-passing kernel for 8 Trainium2 NeuronCores.

Strategy: data-parallel over the batch (structure b -> core b). The graph
(idx1/idx2) is shared across the batch and known at build time, so all
gather/scatter bookkeeping is precomputed on the host and baked into the
kernel as static access patterns + small data tensors.

Per-core device algorithm (N=20000 nodes, E=320000 edges, EMB=8):
  - node table kept in SBUF as [128, N] f32: row 16g+r holds s[:, r] for
    r<8 (replicated for the 8 GPSIMD cores), rows 16g+8.. are junk.
  - edges sorted by idx1, grouped into "windows" (<=512 nodes, <=8192
    edges), each window's edges split into 8 groups of <=1024 (padded).
  - per block: gpsimd.ap_gather pulls s[idx1], s[idx2] into [128, 1024]
    tiles (group g's stream on partitions 16g..16g+15); three fused
    block-diagonal matmuls (s1, s2, gaussian-bond) produce sigmoid and
    softmax-branch pre-activations in PSUM [128, 1024]; ACT sigmoid(+bias)
    and DVE relu(+bias) and DVE mul give v in bf16; PE transposes flip
    edges onto partitions; per-128-edge matmuls with static one-hot rhs
    accumulate node deltas into a PSUM window; deltas go to DRAM and are
    applied to the table at block end (skipped for the last block, where
    only the edge-sum is needed for the mean).
"""

import numpy as np

# ---------------------------------------------------------------- constants
B, N, E = 8, 20000, 320000
EMB = 8
CENTERS = 10
H1 = H2 = 24
N_BLOCKS = 6
MX_D, MN_D, WIDTH = 10.0, 0.0, 1.0
CAT = 3 * EMB  # 24

NGROUP = 8          # gpsimd cores per NeuronCore
TCOLS = 1024        # columns per group per window tile
CAP_E = NGROUP * TCOLS   # max edges per window
CAP_N = 512         # max nodes per window (one PSUM bank)
P = 128

F32 = np.float32
I16 = np.int16


# ---------------------------------------------------------------- host prep
def _prep(idx1: np.ndarray, idx2: np.ndarray, sim_safe: bool = False) -> dict:
    """Sort/partition the graph into windows, groups and scatter chunks."""
    idx1 = np.asarray(idx1, np.int64)
    idx2 = np.asarray(idx2, np.int64)
    order = np.argsort(idx1, kind="stable")
    i1s = idx1[order]
    i2s = idx2[order]

    counts = np.bincount(i1s, minlength=N)
    # greedy windows: consecutive nodes while edges <= CAP_E and nodes <= CAP_N
    win_n0 = []
    win_n1 = []
    n = 0
    while n < N:
        e_acc = 0
        n0 = n
        while n < N and (n - n0) < CAP_N:
            c = int(counts[n])
            if e_acc + c > CAP_E and n > n0:
                break
            e_acc += c
            n += 1
            if e_acc > CAP_E:
                raise RuntimeError("single node exceeds window capacity")
        win_n0.append(n0)
        win_n1.append(n)
    NW = len(win_n0)
    edge_start = np.concatenate([[0], np.cumsum(counts)])

    S = NW * TCOLS  # columns per group

    # esrc[g, c]: original sorted-edge position for (group g, column c), -1 pad
    esrc = np.full((NGROUP, S), -1, np.int64)
    for w in range(NW):
        e0, e1 = int(edge_start[win_n0[w]]), int(edge_start[win_n1[w]])
        ew = e1 - e0
        base, rem = divmod(ew, NGROUP)
        off = e0
        for g in range(NGROUP):
            ln = base + (1 if g < rem else 0)
            esrc[g, w * TCOLS : w * TCOLS + ln] = np.arange(off, off + ln)
            off += ln
        assert off == e1
    pad = esrc < 0

    def streams(vals_sorted):
        return np.where(pad, 0, vals_sorted[np.clip(esrc, 0, None)])

    g1 = streams(i1s)  # [NGROUP, S] destination node per column
    g2 = streams(i2s)

    def wrap(stream):
        # ap_gather layout: index i of core-g stream at [16g + i%16, i//16]
        out = np.zeros((P, S // 16), I16)
        for g in range(NGROUP):
            out[16 * g : 16 * g + 16, :] = stream[g].reshape(S // 16, 16).T
        return out

    idx1w = wrap(g1)
    idx2w = wrap(g2)
    # combined per-window stream: 1024 idx1 cols then 1024 idx2 cols
    comb = np.zeros((P, 2 * S // 16), I16)
    for w in range(NW):
        cs = slice(w * TCOLS, (w + 1) * TCOLS)
        for g in range(NGROUP):
            st = np.concatenate([g1[g, cs], g2[g, cs]])
            comb[16 * g : 16 * g + 16, w * P : (w + 1) * P] = \
                st.reshape(2 * TCOLS // 16, 16).T

    # ---- scatter chunks: (window w, col-range k, group g) of 128 edges
    oh_cols = []           # list of [128, width] float arrays
    chunks = []            # per window: list of (k, g, off, wdt, c0, start)
    oh_total = 0
    for w in range(NW):
        n0, n1 = win_n0[w], win_n1[w]
        nw = n1 - n0
        covered = np.zeros(nw, bool)
        wchunks = []
        first = True
        for k in range(TCOLS // P):
            for g in range(NGROUP):
                cols = slice(w * TCOLS + k * P, w * TCOLS + k * P + P)
                e = esrc[g, cols]
                real = e >= 0
                if not real.any() and not first:
                    continue
                if real.any():
                    loc = g1[g, cols] - n0
                    c0, c1 = int(loc[real].min()), int(loc[real].max()) + 1
                else:
                    loc = np.zeros(P, np.int64)
                    c0, c1 = 0, 1
                oh = np.zeros((P, c1 - c0), F32)
                oh[np.nonzero(real)[0], loc[real] - c0] = 1.0
                oh_cols.append(oh)
                if first:
                    wchunks.append([k, g, oh_total, c1 - c0, c0, True])
                elif not sim_safe:
                    wchunks.append([k, g, oh_total, c1 - c0, c0, False])
                else:
                    # split at written/fresh boundaries so each matmul region
                    # is uniformly accumulated or overwritten (PSUM
                    # has_written is per element; the sim wants uniformity)
                    a = c0
                    while a < c1:
                        st8 = bool(covered[a])
                        b = a
                        while b < c1 and bool(covered[b]) == st8:
                            b += 1
                        wchunks.append([k, g, oh_total + (a - c0), b - a,
                                        a, False])
                        a = b
                covered[c0:c1] = True
                oh_total += c1 - c0
                first = False
        # never-written columns (deg-0 nodes outside all chunk spans):
        # emit zero one-hot chunks per contiguous run so the window PSUM
        # is fully initialized before it is read.
        miss = np.nonzero(~covered)[0]
        if len(miss):
            runs = np.split(miss, np.nonzero(np.diff(miss) != 1)[0] + 1)
            for run in runs:
                oh_cols.append(np.zeros((P, len(run)), F32))
                wchunks.append([0, 0, oh_total, len(run), int(run[0]), False])
                oh_total += len(run)
        chunks.append(wchunks)

    oneh = np.concatenate(oh_cols, axis=1) if oh_cols else np.zeros((P, 0), F32)

    return dict(
        order=order, i1s=i1s, i2s=i2s, esrc=esrc, pad=pad,
        win_n0=np.array(win_n0), win_n1=np.array(win_n1), NW=NW, S=S,
        idx1w=idx1w, idx2w=idx2w, idx12w=comb, oneh=oneh,
        chunks=chunks,
    )


# ------------------------------------------------------- host-built weights
def _consts(pp, W_site, b_site, W_bond, b_bond, W_sig, b_sig, W_sm, b_sm):
    """Build all static lhsT / bias tensors in device layouts."""
    c = {}
    # table init: psum[16g+r, :] = W_site[0, r] * sites
    t = np.zeros((1, P), F32)
    for g in range(NGROUP):
        t[0, 16 * g : 16 * g + EMB] = W_site[0]
    c["lhsT_site"] = t
    bias = np.zeros((P, 1), F32)
    for g in range(NGROUP):
        bias[16 * g : 16 * g + EMB, 0] = b_site
    c["bias_site"] = bias

    # bonds replicate [8 -> 80]
    t = np.zeros((NGROUP, 80), F32)
    for g in range(NGROUP):
        t[g, 10 * g : 10 * g + CENTERS] = 1.0
    c["lhsT_brep"] = t
    cent = np.linspace(MN_D, MX_D, CENTERS, dtype=F32)
    bias = np.zeros((80, 1), F32)
    for g in range(NGROUP):
        bias[10 * g : 10 * g + CENTERS, 0] = -cent
    c["bias_cent"] = bias

    # gaussian -> bond embedding: [80, 64]
    t = np.zeros((80, 64), F32)
    for g in range(NGROUP):
        t[10 * g : 10 * g + CENTERS, 8 * g : 8 * g + EMB] = W_bond
    c["lhsT_wbond"] = t
    bias = np.zeros((64, 1), F32)
    for g in range(NGROUP):
        bias[8 * g : 8 * g + EMB, 0] = b_bond
    c["bias_bond"] = bias

    # per-block lhsT for s1/s2 [128, 128], bnd [64, 128], biases [64, NB]
    bs_sig = np.zeros((64, N_BLOCKS), F32)
    bs_sm = np.zeros((64, N_BLOCKS), F32)
    for i in range(N_BLOCKS):
        for nm, rows in (("s1", slice(0, 8)), ("s2", slice(8, 16))):
            t = np.zeros((P, P), F32)
            for g in range(NGROUP):
                t[16 * g : 16 * g + EMB, 8 * g : 8 * g + EMB] = W_sig[i][rows]
                t[16 * g : 16 * g + EMB, 64 + 8 * g : 64 + 8 * g + EMB] = \
                    W_sm[i][rows]
            c[f"lhsT_{nm}_{i}"] = t
        t = np.zeros((64, P), F32)
        for g in range(NGROUP):
            t[8 * g : 8 * g + EMB, 8 * g : 8 * g + EMB] = W_sig[i][16:24]
            t[8 * g : 8 * g + EMB, 64 + 8 * g : 64 + 8 * g + EMB] = W_sm[i][16:24]
        c[f"lhsT_bnd_{i}"] = t
        for g in range(NGROUP):
            bs_sig[8 * g : 8 * g + EMB, i] = b_sig[i]
            bs_sm[8 * g : 8 * g + EMB, i] = b_sm[i]
    c["bias_sig"] = bs_sig
    c["bias_sm"] = bs_sm

    # transpose identity [64, 64] and replicate matrix [8, 128]
    c["ident64"] = np.eye(64, dtype=F32)
    t = np.zeros((P, P), F32)
    for z in range(3):
        for g in range(NGROUP):
            t[32 * z : 32 * z + EMB, 16 * g : 16 * g + EMB] = \
                np.eye(EMB, dtype=F32)
    c["lhsT_rep"] = t
    t = np.zeros((P, EMB), F32)
    for z in range(3):
        t[32 * z : 32 * z + EMB, :] = np.eye(EMB, dtype=F32)
    c["lhsT_fold"] = t
    t = np.zeros((P, EMB), F32)
    t[0:EMB, :] = np.eye(EMB, dtype=F32)
    c["lhsT_id8"] = t
    return c


# ---------------------------------------------------- numpy model (testing)
def _numpy_model(pp, cn, sites, bonds_g, W1, b1, W2, b2, W3, b3):
    """Exact simulation of the device algorithm (one structure)."""
    import ml_dtypes
    bf16 = ml_dtypes.bfloat16

    def tobf(x):
        return x.astype(bf16).astype(F32)

    S, NW = pp["S"], pp["NW"]
    table = (cn["lhsT_site"].T @ sites[None, :]) + cn["bias_site"]
    brep = cn["lhsT_brep"].T @ bonds_g                      # [80, S]
    gsq = np.square(brep + cn["bias_cent"])
    gb = tobf(np.exp(-gsq))
    bnd64 = tobf(cn["lhsT_wbond"].T @ gb + cn["bias_bond"])  # [64, S]

    def unwrap(w):
        out = np.zeros((NGROUP, S), np.int64)
        for g in range(NGROUP):
            out[g] = w[16 * g : 16 * g + 16, :].T.reshape(-1)
        return out

    st1 = unwrap(pp["idx1w"])
    st2 = unwrap(pp["idx2w"])

    vacc = np.zeros(EMB, F32)
    for i in range(N_BLOCKS):
        delta = np.zeros((EMB, N), F32)
        for w in range(NW):
            cols = slice(w * TCOLS, (w + 1) * TCOLS)
            s1g = np.zeros((P, TCOLS), F32)
            s2g = np.zeros((P, TCOLS), F32)
            for g in range(NGROUP):
                rows = slice(16 * g, 16 * g + 16)
                s1g[rows] = table[rows][:, st1[g, cols]]
                s2g[rows] = table[rows][:, st2[g, cols]]
            pre = (
                cn[f"lhsT_s1_{i}"].T @ s1g
                + cn[f"lhsT_s2_{i}"].T @ s2g
                + cn[f"lhsT_bnd_{i}"].T @ bnd64[:, cols]
            )
            sig = tobf(1.0 / (1.0 + np.exp(-(pre[0:64] + cn["bias_sig"][:, i:i+1]))))
            sm = tobf(np.maximum(pre[64:128] + cn["bias_sm"][:, i:i+1], 0.0))
            v = tobf(sig * sm)                               # [64, TCOLS]
            n0, n1 = pp["win_n0"][w], pp["win_n1"][w]
            psw = np.zeros((EMB, CAP_N), F32)
            for (k, g, off, wdt, c0, start) in pp["chunks"][w]:
                vt = v[8 * g : 8 * g + EMB, k * P : k * P + P]   # [8, 128]
                oh = pp["oneh_f32"][:, off : off + wdt]
                contrib = vt @ oh                                 # [8, wdt]
                if start:
                    psw[:] = 0.0
                psw[:, c0 : c0 + wdt] += contrib
            nw = n1 - n0
            if i == N_BLOCKS - 1:
                vacc += psw[:, :nw].sum(axis=1)
            else:
                delta[:, n0:n1] = psw[:, :nw]
        if i < N_BLOCKS - 1:
            table = table + cn["lhsT_rep"][0:EMB].T @ delta

    tred = table[0:EMB].sum(axis=1)
    vec = (tred + vacc) / N
    h = np.maximum(vec @ W1 + b1, 0.0)
    h = np.maximum(h @ W2 + b2, 0.0)
    return h @ W3 + b3


# ------------------------------------------------------------- bass kernel
def _build_bass(pp):
    import concourse.bass as bass
    import concourse.bacc as bacc
    import concourse.mybir as mybir
    from concourse.tile import TileContext

    AF = mybir.ActivationFunctionType
    ALU = mybir.AluOpType
    f32, bf16, i16 = mybir.dt.float32, mybir.dt.bfloat16, mybir.dt.int16

    S, NW = pp["S"], pp["NW"]
    OH = pp["oneh"].shape[1]
    NT = S // TCOLS  # == NW column tiles per pass

    nc = bacc.Bacc(None, target_bir_lowering=False, debug=False)

    dp = {}

    def param(name, shape, dt):
        dp[name] = nc.declare_dram_parameter(name, list(shape), dt,
                                             isOutput=False)
        return dp[name]

    sites_p = param("sites", (1, N), f32)
    bonds_p = param("bonds_g", (NGROUP, S), f32)
    idx12_p = param("idx12w", (P, 2 * S // 16), i16)
    oneh_p = param("oneh", (P, OH), bf16)
    lhs_site_p = param("lhsT_site", (1, P), f32)
    bias_site_p = param("bias_site", (P, 1), f32)
    lhs_brep_p = param("lhsT_brep", (NGROUP, 80), f32)
    bias_cent_p = param("bias_cent", (80, 1), f32)
    lhs_wbond_p = param("lhsT_wbond", (80, 64), bf16)
    bias_bond_p = param("bias_bond", (64, 1), f32)
    for i in range(N_BLOCKS):
        param(f"lhsT_s1_{i}", (P, P), f32)
        param(f"lhsT_s2_{i}", (P, P), f32)
        param(f"lhsT_bnd_{i}", (64, P), bf16)
    bias_sig_p = param("bias_sig", (64, N_BLOCKS), f32)
    bias_sm_p = param("bias_sm", (64, N_BLOCKS), f32)
    ident_p = param("ident64", (64, 64), bf16)
    lhs_rep_p = param("lhsT_rep", (P, P), f32)
    w1_p = param("W1", (EMB, H1), f32)
    b1_p = param("b1", (H1, 1), f32)
    w2_p = param("W2", (H1, H2), f32)
    b2_p = param("b2", (H2, 1), f32)
    fold_p = param("lhsT_fold", (P, EMB), f32)
    id8_p = param("lhsT_id8", (P, EMB), f32)
    w3_p = param("W3", (H2, 1), f32)
    b3_p = param("b3", (1, 1), f32)
    out_p = nc.declare_dram_parameter("out", [1, 1], f32, isOutput=True)

    with TileContext(nc) as tc:
        with (
            tc.tile_pool(name="const", bufs=1) as cp,
            tc.tile_pool(name="work", bufs=2) as wp,
            tc.tile_pool(name="dram", bufs=1, space="DRAM") as dr,
            tc.tile_pool(name="psum", bufs=2, space="PSUM") as pp_pre,
            tc.tile_pool(name="psum_t", bufs=2, space="PSUM") as pp_t,
            tc.tile_pool(name="psum_w", bufs=1, space="PSUM") as pp_w,
            tc.tile_pool(name="psum_wb", bufs=1, space="PSUM") as pp_wb,
        ):
            bnd_dram = dr.tile([64, S], bf16, tag="bnd_dram")
            delta_dram = dr.tile([P, NW * CAP_N], f32, tag="delta_dram")

            # ------- persistent SBUF tensors
            table = cp.tile([P, N], f32, tag="table")
            oneh = cp.tile([P, OH], bf16, tag="oneh")
            idx12w = cp.tile([P, 2 * S // 16], i16, tag="idx12w")
            vacc = cp.tile([P, NW + 1], f32, tag="vacc")

            def cload(prm, shape, dt, tag):
                t = cp.tile(list(shape), dt, tag=tag)
                nc.sync.dma_start(out=t[:], in_=prm[:])
                return t

            nc.sync.dma_start(out=oneh[:], in_=oneh_p[:])
            nc.sync.dma_start(out=idx12w[:], in_=idx12_p[:])
            lhs_site = cload(lhs_site_p, (1, P), f32, "lhs_site")
            bias_site = cload(bias_site_p, (P, 1), f32, "bias_site")
            lhs_brep = cload(lhs_brep_p, (NGROUP, 80), f32, "lhs_brep")
            bias_cent = cload(bias_cent_p, (80, 1), f32, "bias_cent")
            lhs_wbond = cload(lhs_wbond_p, (80, 64), bf16, "lhs_wbond")
            bias_bond = cload(bias_bond_p, (64, 1), f32, "bias_bond")
            ident64 = cload(ident_p, (64, 64), bf16, "ident64")
            lhs_rep = cload(lhs_rep_p, (P, P), f32, "lhs_rep")
            bias_sig = cload(bias_sig_p, (64, N_BLOCKS), f32, "bias_sig")
            bias_sm = cload(bias_sm_p, (64, N_BLOCKS), f32, "bias_sm")
            blk_c = []
            for i in range(N_BLOCKS):
                blk_c.append((
                    cload(dp[f"lhsT_s1_{i}"], (P, P), f32, f"lhs_s1_{i}"),
                    cload(dp[f"lhsT_s2_{i}"], (P, P), f32, f"lhs_s2_{i}"),
                    cload(dp[f"lhsT_bnd_{i}"], (64, P), bf16, f"lhs_bnd_{i}"),
                ))
            w1 = cload(w1_p, (EMB, H1), f32, "w1")
            b1 = cload(b1_p, (H1, 1), f32, "b1")
            w2 = cload(w2_p, (H1, H2), f32, "w2")
            b2 = cload(b2_p, (H2, 1), f32, "b2")
            lhs_fold = cload(fold_p, (P, EMB), f32, "lhs_fold")
            lhs_id8 = cload(id8_p, (P, EMB), f32, "lhs_id8")
            w3 = cload(w3_p, (H2, 1), f32, "w3")
            b3 = cload(b3_p, (1, 1), f32, "b3")

            # ------- phase A1: table init
            ACOLS = 512
            wpa_cm = tc.tile_pool(name="worka", bufs=2)
            wpa = wpa_cm.__enter__()
            for j in range((N + ACOLS - 1) // ACOLS):
                c0 = j * ACOLS
                c1 = min(N, c0 + ACOLS)
                w = c1 - c0
                st = wpa.tile([1, ACOLS], f32, tag="sites")
                nc.sync.dma_start(out=st[:, :w], in_=sites_p[:, c0:c1])
                ps = pp_t.tile([P, 512], f32, tag="pt")
                nc.tensor.matmul(out=ps[:, :w], lhsT=lhs_site[:],
                                 rhs=st[:, :w], start=True, stop=True)
                nc.scalar.activation(table[:, c0:c1], ps[:, :w], AF.Identity,
                                     bias=bias_site[:])

            # ------- phase A2: bnd embedding build -> DRAM
            for j in range(S // ACOLS):
                c0 = j * ACOLS
                bt = wpa.tile([NGROUP, ACOLS], f32, tag="bondsin")
                nc.sync.dma_start(out=bt[:], in_=bonds_p[:, c0:c0 + ACOLS])
                ps = pp_t.tile([P, 512], f32, tag="pt")
                nc.tensor.matmul(out=ps[:80, :], lhsT=lhs_brep[:],
                                 rhs=bt[:], start=True, stop=True)
                gsq = wpa.tile([80, ACOLS], f32, tag="gsq")
                nc.scalar.activation(gsq[:], ps[:80, :], AF.Square,
                                     bias=bias_cent[:])
                gb = wpa.tile([80, ACOLS], bf16, tag="gb")
                nc.scalar.activation(gb[:], gsq[:], AF.Exp, scale=-1.0)
                bt2 = wpa.tile([64, ACOLS], bf16, tag="bnd64")
                ps2 = pp_t.tile([P, 512], f32, tag="pt")
                nc.tensor.matmul(out=ps2[:64, :], lhsT=lhs_wbond[:],
                                 rhs=gb[:], start=True, stop=True)
                nc.scalar.activation(bt2[:], ps2[:64, :], AF.Identity,
                                     bias=bias_bond[:])
                nc.sync.dma_start(out=bnd_dram[:, c0:c0 + ACOLS], in_=bt2[:])

            wpa_cm.__exit__(None, None, None)
            nc.vector.memset(vacc[:], 0.0)
            zerosP = cp.tile([P, P], bf16, tag="zerosP")
            zeros128 = cp.tile([P, CAP_N], bf16, tag="zeros128")
            nc.vector.memset(zerosP[:], 0.0)
            nc.vector.memset(zeros128[:], 0.0)

            # ------- phase B: conv blocks (software-pipelined: the compute
            # stage of window w overlaps the scatter stage of window w-1,
            # and the scatter accumulates into two separate PSUM banks to
            # break the per-bank read-modify-write chain)
            for i in range(N_BLOCKS):
                last = i == N_BLOCKS - 1
                l_s1, l_s2, l_bnd = blk_c[i]
                state = {}

                def compute_stage(w):
                    c0 = w * TCOLS
                    s12g = wp.tile([P, 2 * TCOLS], f32, tag="s12g")
                    nc.gpsimd.ap_gather(
                        s12g[:], table[:],
                        idx12w[:, w * P : (w + 1) * P],
                        channels=P, num_elems=N, d=1, num_idxs=2 * TCOLS)
                    bnd_t = wp.tile([64, TCOLS], bf16, tag="bnd_t")
                    nc.sync.dma_start(out=bnd_t[:],
                                      in_=bnd_dram[:, c0:c0 + TCOLS])
                    ps = pp_pre.tile([P, TCOLS], f32, tag="pre")
                    for h in range(2):
                        hs = slice(512 * h, 512 * h + 512)
                        nc.tensor.matmul(out=ps[:, hs], lhsT=l_s1[:],
                                         rhs=s12g[:, hs], start=True,
                                         stop=False)
                        nc.tensor.matmul(
                            out=ps[:, hs], lhsT=l_s2[:],
                            rhs=s12g[:, 1024:][:, hs], start=False,
                            stop=False)
                        nc.tensor.matmul(out=ps[:, hs], lhsT=l_bnd[:],
                                         rhs=bnd_t[:, hs], start=False,
                                         stop=True)
                    sig = wp.tile([64, TCOLS], bf16, tag="sig")
                    nc.scalar.activation(sig[:], ps[0:64, :], AF.Sigmoid,
                                         bias=bias_sig[:, i : i + 1])
                    v = wp.tile([64, TCOLS], bf16, tag="v")
                    nc.vector.tensor_scalar(
                        out=v[:], in0=ps[64:128, :],
                        scalar1=bias_sm[:, i : i + 1], scalar2=0.0,
                        op0=ALU.add, op1=ALU.max)
                    nc.vector.tensor_mul(v[:], v[:], sig[:])
                    vts = {}
                    for kk in range(2):
                        pst = pp_t.tile([P, 512], bf16, tag="pt")
                        for k4 in range(4):
                            k = 4 * kk + k4
                            nc.tensor.matmul(
                                out=pst[:, 64 * k4 : 64 * k4 + 64],
                                lhsT=v[:, k * P : k * P + P], rhs=ident64[:],
                                is_transpose=True, start=(k4 == 0),
                                stop=(k4 == 3))
                        vt = wp.tile([P, 256], bf16, tag="vt")
                        nc.scalar.activation(vt[:], pst[:, :256], AF.Copy)
                        vts[kk] = vt
                    state[w] = vts

                def scatter_stage(w):
                    vts = state.pop(w)
                    n0 = int(pp["win_n0"][w])
                    n1 = int(pp["win_n1"][w])
                    nw = n1 - n0
                    psw = pp_w.tile([P, CAP_N], f32, tag="win")
                    nc.tensor.matmul(
                        out=psw[:, 0:nw], lhsT=zerosP[:],
                        rhs=zeros128[:, 0:nw], start=True, stop=False,
                        skip_group_check=True)
                    nch = len(pp["chunks"][w])
                    for ci, (k, g, off, wdt, cc0, start) in \
                            enumerate(pp["chunks"][w]):
                        vt = vts[k // 4]
                        z = 32 * (ci % 3)
                        nc.tensor.matmul(
                            out=psw[z : z + EMB, cc0 : cc0 + wdt],
                            lhsT=vt[:, 64 * (k % 4) + 8 * g :
                                    64 * (k % 4) + 8 * g + EMB],
                            rhs=oneh[:, off : off + wdt],
                            start=False, stop=(ci == nch - 1),
                            skip_group_check=True)
                    if last:
                        nc.vector.tensor_reduce(
                            vacc[:, w : w + 1], psw[:, :nw],
                            axis=mybir.AxisListType.X, op=ALU.add)
                    else:
                        d8o = wp.tile([P, CAP_N], f32, tag="d8o")
                        nc.scalar.activation(d8o[:, :nw], psw[:, :nw],
                                             AF.Copy)
                        nc.sync.dma_start(
                            out=delta_dram[:, w * CAP_N : w * CAP_N + nw],
                            in_=d8o[:, :nw])

                for w in range(NW + 1):
                    if w < NW:
                        compute_stage(w)
                    if w >= 1:
                        scatter_stage(w - 1)
                if not last:
                    for w in range(NW):
                        n0 = int(pp["win_n0"][w])
                        n1 = int(pp["win_n1"][w])
                        nw = n1 - n0
                        d8 = wp.tile([P, CAP_N], f32, tag="d8")
                        nc.sync.dma_start(
                            out=d8[:, :nw],
                            in_=delta_dram[:, w * CAP_N : w * CAP_N + nw])
                        psr = pp_t.tile([P, 512], f32, tag="pt")
                        nc.tensor.matmul(out=psr[:, :nw], lhsT=lhs_rep[:],
                                         rhs=d8[:, :nw], start=True,
                                         stop=True)
                        nc.vector.tensor_add(table[:, n0:n1],
                                             table[:, n0:n1], psr[:, :nw])

            # ------- phase C: mean + MLP
            tred = cp.tile([P, 2], f32, tag="tred")
            nc.vector.tensor_reduce(tred[0:EMB, 0:1], table[0:EMB, :],
                                    axis=mybir.AxisListType.X, op=ALU.add)
            nc.vector.tensor_reduce(tred[:, 1:2], vacc[:, :NW],
                                    axis=mybir.AxisListType.X, op=ALU.add)
            # fold the three staggered row-sets + table sum on PE, then
            # scale by 1/N on the way out of PSUM
            psv = pp_t.tile([P, 512], f32, tag="pt")
            nc.tensor.matmul(out=psv[:EMB, 0:1], lhsT=lhs_fold[:],
                             rhs=tred[:, 1:2], start=True, stop=False)
            nc.tensor.matmul(out=psv[:EMB, 0:1], lhsT=lhs_id8[:],
                             rhs=tred[:, 0:1], start=False, stop=True)
            vec = cp.tile([EMB, 1], f32, tag="vec")
            nc.scalar.activation(vec[:], psv[:EMB, 0:1], AF.Identity,
                                 scale=1.0 / N)
            psm = pp_t.tile([P, 512], f32, tag="pt")
            nc.tensor.matmul(out=psm[:H1, 0:1], lhsT=w1[:], rhs=vec[:],
                             start=True, stop=True)
            h1t = cp.tile([H1, 1], f32, tag="h1")
            nc.scalar.activation(h1t[:], psm[:H1, 0:1], AF.Relu, bias=b1[:])
            psm2 = pp_t.tile([P, 512], f32, tag="pt")
            nc.tensor.matmul(out=psm2[:H2, 0:1], lhsT=w2[:], rhs=h1t[:],
                             start=True, stop=True)
            h2t = cp.tile([H2, 1], f32, tag="h2")
            nc.scalar.activation(h2t[:], psm2[:H2, 0:1], AF.Relu, bias=b2[:])
            psm3 = pp_t.tile([P, 512], f32, tag="pt")
            nc.tensor.matmul(out=psm3[:1, 0:1], lhsT=w3[:], rhs=h2t[:],
                             start=True, stop=True)
            ot = cp.tile([1, 1], f32, tag="ot")
            nc.scalar.activation(ot[:], psm3[:1, 0:1], AF.Identity,
                                 bias=b3[:])
            nc.sync.dma_start(out=out_p[:], in_=ot[:])

    nc.compile()
    return nc


def _in_maps(pp, cn, sites, bonds, mlp):
    import ml_dtypes
    bf16 = ml_dtypes.bfloat16
    shared = {
        "idx12w": pp["idx12w"],
        "oneh": pp["oneh"].astype(bf16),
    }
    for k, v in cn.items():
        if k == "ident64" or k == "lhsT_wbond" or k.startswith("lhsT_bnd_"):
            shared[k] = v.astype(bf16)
        else:
            shared[k] = v.astype(F32)
    shared.update(mlp)
    in_maps = []
    esrc = pp["esrc"]
    for b in range(B):
        bsorted = bonds[b, :, 0][pp["order"]]
        bg = np.where(pp["pad"], 0.0, bsorted[np.clip(esrc, 0, None)])
        m = dict(shared)
        m["sites"] = np.ascontiguousarray(sites[b, :, 0][None, :], F32)
        m["bonds_g"] = bg.astype(F32)
        in_maps.append(m)
    return in_maps


def kernel(sites, bonds, idx1, idx2, W_site, b_site, W_bond, b_bond,
           W_sig, b_sig, W_sm, b_sm, W1, b1, W2, b2, W3, b3):
    sites = np.asarray(sites, F32)
    bonds = np.asarray(bonds, F32)
    pp = _prep(np.asarray(idx1), np.asarray(idx2))
    cn = _consts(pp, np.asarray(W_site, F32), np.asarray(b_site, F32),
                 np.asarray(W_bond, F32), np.asarray(b_bond, F32),
                 np.asarray(W_sig, F32), np.asarray(b_sig, F32),
                 np.asarray(W_sm, F32), np.asarray(b_sm, F32))
    mlp = {
        "W1": np.asarray(W1, F32), "b1": np.asarray(b1, F32)[:, None],
        "W2": np.asarray(W2, F32), "b2": np.asarray(b2, F32)[:, None],
        "W3": np.asarray(W3, F32), "b3": np.asarray(b3, F32)[:, None],
    }
    nc = _build_bass(pp)
    in_maps = _in_maps(pp, cn, sites, bonds, mlp)
    from concourse.bass_utils import run_bass_kernel_spmd
    res = run_bass_kernel_spmd(nc, in_maps, list(range(B)))
    global LAST_RESULT
    LAST_RESULT = res
    out = np.stack([np.asarray(res.results[b]["out"]).reshape(1)
                    for b in range(B)], axis=0)
    return out.astype(F32)


LAST_RESULT = None



# revision 20
# speedup vs baseline: 1.5105x; 1.5105x over previous
"""CGCNN message-passing kernel for 8 Trainium2 NeuronCores.

Strategy: data-parallel over the batch (structure b -> core b). The graph
(idx1/idx2) is shared across the batch and known at build time, so all
gather/scatter bookkeeping is precomputed on the host and baked into the
kernel as static access patterns + small data tensors.

Per-core device algorithm (N=20000 nodes, E=320000 edges, EMB=8):
  - node table kept in SBUF as [128, N] f32: row 16g+r holds s[:, r] for
    r<8 (replicated for the 8 GPSIMD cores), rows 16g+8.. are junk.
  - edges sorted by idx1, grouped into "windows" (<=512 nodes, <=8192
    edges), each window's edges split into 8 groups of <=1024 (padded).
  - per block: gpsimd.ap_gather pulls s[idx1], s[idx2] into [128, 1024]
    tiles (group g's stream on partitions 16g..16g+15); three fused
    block-diagonal matmuls (s1, s2, gaussian-bond) produce sigmoid and
    softmax-branch pre-activations in PSUM [128, 1024]; ACT sigmoid(+bias)
    and DVE relu(+bias) and DVE mul give v in bf16; PE transposes flip
    edges onto partitions; per-128-edge matmuls with static one-hot rhs
    accumulate node deltas into a PSUM window; deltas go to DRAM and are
    applied to the table at block end (skipped for the last block, where
    only the edge-sum is needed for the mean).
"""

import numpy as np

# ---------------------------------------------------------------- constants
B, N, E = 8, 20000, 320000
EMB = 8
CENTERS = 10
H1 = H2 = 24
N_BLOCKS = 6
MX_D, MN_D, WIDTH = 10.0, 0.0, 1.0
CAT = 3 * EMB  # 24

NGROUP = 8          # gpsimd cores per NeuronCore
TCOLS = 1024        # columns per group per window tile
CAP_E = NGROUP * TCOLS   # max edges per window
CAP_N = 512         # max nodes per window (one PSUM bank)
P = 128

F32 = np.float32
I16 = np.int16


# ---------------------------------------------------------------- host prep
def _prep(idx1: np.ndarray, idx2: np.ndarray, sim_safe: bool = False) -> dict:
    """Sort/partition the graph into windows, groups and scatter chunks.

    Nodes are renumbered block-major: new id = 128*j + 16*g + u, where the
    128-node block j is dealt snake-wise from the degree-sorted node list so
    every group's 16-node slice of a block has a near-equal edge count.
    Group g's stream holds edges whose (new) dest is in rows [16g,16g+16)
    of a block; a window is 1-4 consecutive blocks.
    """
    idx1 = np.asarray(idx1, np.int64)
    idx2 = np.asarray(idx2, np.int64)

    deg = np.bincount(idx1, minlength=N)
    byd = np.argsort(-deg, kind="stable")
    NB = (N + P - 1) // P
    newid = np.zeros(N, np.int64)
    for j in range(NB):
        blk = byd[j * P : (j + 1) * P]
        for k, old in enumerate(blk):
            newid[old] = P * j + 16 * (k % NGROUP) + (k // NGROUP)
    idx1 = newid[idx1]
    idx2 = newid[idx2]

    order = np.argsort(idx1, kind="stable")
    i1s = idx1[order]
    i2s = idx2[order]

    counts = np.bincount(i1s, minlength=NB * P)
    edge_start = np.concatenate([[0], np.cumsum(counts)])
    # per (block, group) edge count and block width = max over groups
    cnt_bg = counts.reshape(NB, NGROUP, 16).sum(axis=2)
    width_b = cnt_bg.max(axis=1)

    # windows: consecutive blocks, <=4 blocks, <=TCOLS columns
    win_b0, win_b1 = [], []
    j = 0
    while j < NB:
        b0 = j
        wcols = 0
        while j < NB and (j - b0) < 4 and wcols + width_b[j] <= TCOLS:
            wcols += width_b[j]
            j += 1
        if j == b0:
            raise RuntimeError("block exceeds window capacity")
        win_b0.append(b0)
        win_b1.append(j)
    NW = len(win_b0)
    win_n0 = [b0 * P for b0 in win_b0]
    win_n1 = [b1 * P for b1 in win_b1]

    S = NW * TCOLS  # columns per group

    # esrc[g, c] + per-window block column ranges (segments)
    esrc = np.full((NGROUP, S), -1, np.int64)
    segs = []  # per window: list of (c0, c1, block j)
    for w in range(NW):
        col = 0
        wsegs = []
        for j in range(win_b0[w], win_b1[w]):
            wd = int(width_b[j])
            if wd > 0:
                wsegs.append((col, col + wd, j))
            for g in range(NGROUP):
                e0 = int(edge_start[P * j + 16 * g])
                ln = int(cnt_bg[j, g])
                esrc[g, w * TCOLS + col : w * TCOLS + col + ln] = \
                    np.arange(e0, e0 + ln)
            col += wd
        segs.append(wsegs)
    pad = esrc < 0

    def streams(vals_sorted):
        return np.where(pad, 0, vals_sorted[np.clip(esrc, 0, None)])

    g1 = streams(i1s)  # [NGROUP, S] destination node per column
    g2 = streams(i2s)

    def wrap(stream):
        # ap_gather layout: index i of core-g stream at [16g + i%16, i//16]
        out = np.zeros((P, S // 16), I16)
        for g in range(NGROUP):
            out[16 * g : 16 * g + 16, :] = stream[g].reshape(S // 16, 16).T
        return out

    idx1w = wrap(g1)
    idx2w = wrap(g2)
    # s2-only per-window stream (s1 is produced by expansion matmuls)
    comb = np.zeros((P, S // 16), I16)
    for w in range(NW):
        cs = slice(w * TCOLS, (w + 1) * TCOLS)
        for g in range(NGROUP):
            comb[16 * g : 16 * g + 16, w * 64 : (w + 1) * 64] = \
                g2[g, cs].reshape(TCOLS // 16, 16).T

    # ---- s1 expansion 8-hot rhs [128, NW*TCOLS]: column (w*1024+c) has a
    # one at row (dest & 127) for each group with a real edge there (eight
    # distinct rows since dest = 128j + 16g + u).
    oh_cols_exp = np.zeros((P, NW * TCOLS), np.float32)
    for g in range(NGROUP):
        real = ~pad[g]
        cols = np.nonzero(real)[0]
        oh_cols_exp[g1[g, cols] & (P - 1), cols] = 1.0

    # ---- scatter chunks: (window w, col-range k, group g) of 128 edges
    oh_cols = []           # list of [128, width] float arrays
    chunks = []            # per window: list of (k, g, off, wdt, c0, start)
    oh_total = 0
    for w in range(NW):
        n0, n1 = win_n0[w], win_n1[w]
        nw = n1 - n0
        covered = np.zeros(nw, bool)
        wchunks = []
        first = True
        for k in range(TCOLS // P):
            for g in range(NGROUP):
                cols = slice(w * TCOLS + k * P, w * TCOLS + k * P + P)
                e = esrc[g, cols]
                real = e >= 0
                if not real.any() and not first:
                    continue
                if real.any():
                    loc = g1[g, cols] - n0
                    c0, c1 = int(loc[real].min()), int(loc[real].max()) + 1
                else:
                    loc = np.zeros(P, np.int64)
                    c0, c1 = 0, 1
                oh = np.zeros((P, c1 - c0), F32)
                oh[np.nonzero(real)[0], loc[real] - c0] = 1.0
                oh_cols.append(oh)
                if first:
                    wchunks.append([k, g, oh_total, c1 - c0, c0, True])
                elif not sim_safe:
                    wchunks.append([k, g, oh_total, c1 - c0, c0, False])
                else:
                    # split at written/fresh boundaries so each matmul region
                    # is uniformly accumulated or overwritten (PSUM
                    # has_written is per element; the sim wants uniformity)
                    a = c0
                    while a < c1:
                        st8 = bool(covered[a])
                        b = a
                        while b < c1 and bool(covered[b]) == st8:
                            b += 1
                        wchunks.append([k, g, oh_total + (a - c0), b - a,
                                        a, False])
                        a = b
                covered[c0:c1] = True
                oh_total += c1 - c0
                first = False
        # never-written columns (deg-0 nodes outside all chunk spans):
        # emit zero one-hot chunks per contiguous run so the window PSUM
        # is fully initialized before it is read.
        miss = np.nonzero(~covered)[0]
        if len(miss):
            runs = np.split(miss, np.nonzero(np.diff(miss) != 1)[0] + 1)
            for run in runs:
                oh_cols.append(np.zeros((P, len(run)), F32))
                wchunks.append([0, 0, oh_total, len(run), int(run[0]), False])
                oh_total += len(run)
        chunks.append(wchunks)

    oneh = np.concatenate(oh_cols, axis=1) if oh_cols else np.zeros((P, 0), F32)

    oh_woff = [wch[0][2] for wch in chunks] + [oneh.shape[1]]
    return dict(
        order=order, i1s=i1s, i2s=i2s, esrc=esrc, pad=pad,
        win_n0=np.array(win_n0), win_n1=np.array(win_n1), NW=NW, S=S,
        idx1w=idx1w, idx2w=idx2w, idx12w=comb, oneh=oneh,
        chunks=chunks, oh_exp=oh_cols_exp, segs=segs, NB=NB,
        newid=newid, oh_woff=oh_woff, NV=NB * P,
    )


# ------------------------------------------------------- host-built weights
def _consts(pp, W_site, b_site, W_bond, b_bond, W_sig, b_sig, W_sm, b_sm):
    """Build all static lhsT / bias tensors in device layouts."""
    c = {}
    # table init: psum[16g+r, :] = W_site[0, r] * sites
    t = np.zeros((1, P), F32)
    for g in range(NGROUP):
        t[0, 16 * g : 16 * g + EMB] = W_site[0]
    c["lhsT_site"] = t
    bias = np.zeros((P, 1), F32)
    for g in range(NGROUP):
        bias[16 * g : 16 * g + EMB, 0] = b_site
    c["bias_site"] = bias

    # bonds replicate [8 -> 80]
    t = np.zeros((NGROUP, 80), F32)
    for g in range(NGROUP):
        t[g, 10 * g : 10 * g + CENTERS] = 1.0
    c["lhsT_brep"] = t
    cent = np.linspace(MN_D, MX_D, CENTERS, dtype=F32)
    bias = np.zeros((80, 1), F32)
    for g in range(NGROUP):
        bias[10 * g : 10 * g + CENTERS, 0] = -cent
    c["bias_cent"] = bias

    # gaussian -> bond embedding: [80, 64]
    t = np.zeros((80, 64), F32)
    for g in range(NGROUP):
        t[10 * g : 10 * g + CENTERS, 8 * g : 8 * g + EMB] = W_bond
    c["lhsT_wbond"] = t
    bias = np.zeros((64, 1), F32)
    for g in range(NGROUP):
        bias[8 * g : 8 * g + EMB, 0] = b_bond
    c["bias_bond"] = bias

    # per-block lhsT for s2 [128, 128], bnd [64, 128], biases [64, NB];
    # s1 is folded into T2W (tableT @ W) rebuilt on-device per block
    bs_sig = np.zeros((64, N_BLOCKS), F32)
    bs_sm = np.zeros((64, N_BLOCKS), F32)
    for i in range(N_BLOCKS):
        for nm, rows in (("s2", slice(8, 16)),):
            t = np.zeros((P, P), F32)
            for g in range(NGROUP):
                t[16 * g : 16 * g + EMB, 8 * g : 8 * g + EMB] = W_sig[i][rows]
                t[16 * g : 16 * g + EMB, 64 + 8 * g : 64 + 8 * g + EMB] = \
                    W_sm[i][rows]
            c[f"lhsT_{nm}_{i}"] = t
        c[f"WsigR_{i}"] = np.tile(np.ascontiguousarray(W_sig[i][0:EMB], F32),
                                  (1, NGROUP))
        c[f"WsmR_{i}"] = np.tile(np.ascontiguousarray(W_sm[i][0:EMB], F32),
                                 (1, NGROUP))
        t = np.zeros((64, P), F32)
        for g in range(NGROUP):
            t[8 * g : 8 * g + EMB, 8 * g : 8 * g + EMB] = W_sig[i][16:24]
            t[8 * g : 8 * g + EMB, 64 + 8 * g : 64 + 8 * g + EMB] = W_sm[i][16:24]
        c[f"lhsT_bnd_{i}"] = t
        for g in range(NGROUP):
            bs_sig[8 * g : 8 * g + EMB, i] = b_sig[i]
            bs_sm[8 * g : 8 * g + EMB, i] = b_sm[i]
    c["bias_sig"] = bs_sig
    c["bias_sm"] = bs_sm

    mbd = np.zeros((P, P), F32)
    for g in range(NGROUP):
        for u in range(16):
            mbd[16 * g + u, 8 * g : 8 * g + 8] = 1.0
            mbd[16 * g + u, 64 + 8 * g : 64 + 8 * g + 8] = 1.0
    c["maskBD"] = mbd
    # transpose identity [64, 64] and replicate matrix [8, 128]
    c["ident64"] = np.eye(64, dtype=F32)
    t = np.zeros((P, P), F32)
    for z in range(3):
        for g in range(NGROUP):
            t[32 * z : 32 * z + EMB, 16 * g : 16 * g + EMB] = \
                np.eye(EMB, dtype=F32)
    c["lhsT_rep"] = t
    t = np.zeros((P, EMB), F32)
    for z in range(3):
        t[32 * z : 32 * z + EMB, :] = np.eye(EMB, dtype=F32)
    c["lhsT_fold"] = t
    t = np.zeros((P, EMB), F32)
    t[0:EMB, :] = np.eye(EMB, dtype=F32)
    c["lhsT_id8"] = t
    return c


# ---------------------------------------------------- numpy model (testing)
def _numpy_model(pp, cn, sites, bonds_g, W1, b1, W2, b2, W3, b3):
    """Exact simulation of the device algorithm (one structure)."""
    import ml_dtypes
    bf16 = ml_dtypes.bfloat16

    def tobf(x):
        return x.astype(bf16).astype(F32)

    S, NW = pp["S"], pp["NW"]
    table = (cn["lhsT_site"].T @ sites[None, :]) + cn["bias_site"]
    brep = cn["lhsT_brep"].T @ bonds_g                      # [80, S]
    gsq = np.square(brep + cn["bias_cent"])
    gb = tobf(np.exp(-gsq))
    bnd64 = tobf(cn["lhsT_wbond"].T @ gb + cn["bias_bond"])  # [64, S]

    def unwrap(w):
        out = np.zeros((NGROUP, S), np.int64)
        for g in range(NGROUP):
            out[g] = w[16 * g : 16 * g + 16, :].T.reshape(-1)
        return out

    st1 = unwrap(pp["idx1w"])
    st2 = unwrap(pp["idx2w"])

    vacc = np.zeros(EMB, F32)
    for i in range(N_BLOCKS):
        # T2W: bf16(tableT @ W) per node, s1 contribution via one-hot
        t2wsig = tobf(table[0:EMB].T @ cn[f"WsigR_{i}"][:, 0:EMB])  # [NV, 8]
        t2wsm = tobf(table[0:EMB].T @ cn[f"WsmR_{i}"][:, 0:EMB])
        delta = np.zeros((EMB, table.shape[1]), F32)
        for w in range(NW):
            cols = slice(w * TCOLS, (w + 1) * TCOLS)
            s2g = np.zeros((P, TCOLS), F32)
            for g in range(NGROUP):
                rows = slice(16 * g, 16 * g + 16)
                s2g[rows] = table[rows][:, st2[g, cols]]
            pre = (
                tobf(cn[f"lhsT_s2_{i}"]).T @ tobf(s2g)
                + cn[f"lhsT_bnd_{i}"].T @ bnd64[:, cols]
            )
            wpad = pp["pad"][:, cols]
            for g in range(NGROUP):
                sel = ~wpad[g]
                n1w = st1[g, cols][sel]
                pre[8 * g : 8 * g + EMB, sel] += t2wsig[n1w].T
                pre[64 + 8 * g : 64 + 8 * g + EMB, sel] += t2wsm[n1w].T
            sig = tobf(1.0 / (1.0 + np.exp(-(pre[0:64] + cn["bias_sig"][:, i:i+1]))))
            sm = tobf(np.maximum(pre[64:128] + cn["bias_sm"][:, i:i+1], 0.0))
            v = tobf(sig * sm)                               # [64, TCOLS]
            n0, n1 = pp["win_n0"][w], pp["win_n1"][w]
            psw = np.zeros((EMB, CAP_N), F32)
            for (k, g, off, wdt, c0, start) in pp["chunks"][w]:
                vt = v[8 * g : 8 * g + EMB, k * P : k * P + P]   # [8, 128]
                oh = pp["oneh_f32"][:, off : off + wdt]
                contrib = vt @ oh                                 # [8, wdt]
                if start:
                    psw[:] = 0.0
                psw[:, c0 : c0 + wdt] += contrib
            nw = n1 - n0
            if i == N_BLOCKS - 1:
                vacc += psw[:, :nw].sum(axis=1)
            else:
                delta[:, n0:n1] = psw[:, :nw]
        if i < N_BLOCKS - 1:
            table = table + cn["lhsT_rep"][0:EMB].T @ delta

    tred = table[0:EMB].sum(axis=1)
    vec = (tred + vacc) / N
    h = np.maximum(vec @ W1 + b1, 0.0)
    h = np.maximum(h @ W2 + b2, 0.0)
    return h @ W3 + b3


# ------------------------------------------------------------- bass kernel
def _build_bass(pp):
    import concourse.bass as bass
    import concourse.bacc as bacc
    import concourse.mybir as mybir
    from concourse.tile import TileContext

    AF = mybir.ActivationFunctionType
    ALU = mybir.AluOpType
    f32, bf16, i16 = mybir.dt.float32, mybir.dt.bfloat16, mybir.dt.int16

    S, NW = pp["S"], pp["NW"]
    OH = pp["oneh"].shape[1]
    MAXW = max(pp["oh_woff"][w + 1] - pp["oh_woff"][w] for w in range(NW))
    NT = S // TCOLS  # == NW column tiles per pass

    nc = bacc.Bacc(None, target_bir_lowering=False, debug=False)

    dp = {}

    def param(name, shape, dt):
        dp[name] = nc.declare_dram_parameter(name, list(shape), dt,
                                             isOutput=False)
        return dp[name]

    NB = pp["NB"]
    NV = pp["NV"]
    sites_p = param("sites", (1, NV), f32)
    bonds_p = param("bonds_g", (NGROUP, S), f32)
    idx12_p = param("idx12w", (P, S // 16), i16)
    oneh_p = param("oneh", (P, OH), bf16)
    ohexp_p = param("oh_exp", (P, NW * TCOLS), bf16)
    lhs_site_p = param("lhsT_site", (1, P), f32)
    bias_site_p = param("bias_site", (P, 1), f32)
    lhs_brep_p = param("lhsT_brep", (NGROUP, 80), f32)
    bias_cent_p = param("bias_cent", (80, 1), f32)
    lhs_wbond_p = param("lhsT_wbond", (80, 64), bf16)
    bias_bond_p = param("bias_bond", (64, 1), f32)
    for i in range(N_BLOCKS):
        param(f"lhsT_s2_{i}", (P, P), bf16)
        param(f"lhsT_bnd_{i}", (64, P), bf16)
        param(f"WsigR_{i}", (EMB, 64), f32)
        param(f"WsmR_{i}", (EMB, 64), f32)
    bias_sig_p = param("bias_sig", (64, N_BLOCKS), f32)
    bias_sm_p = param("bias_sm", (64, N_BLOCKS), f32)
    ident_p = param("ident64", (64, 64), bf16)
    maskbd_p = param("maskBD", (P, P), f32)
    lhs_rep_p = param("lhsT_rep", (P, P), f32)
    w1_p = param("W1", (EMB, H1), f32)
    b1_p = param("b1", (H1, 1), f32)
    w2_p = param("W2", (H1, H2), f32)
    b2_p = param("b2", (H2, 1), f32)
    fold_p = param("lhsT_fold", (P, EMB), f32)
    id8_p = param("lhsT_id8", (P, EMB), f32)
    w3_p = param("W3", (H2, 1), f32)
    b3_p = param("b3", (1, 1), f32)
    out_p = nc.declare_dram_parameter("out", [1, 1], f32, isOutput=True)

    with TileContext(nc) as tc:
        with (
            tc.tile_pool(name="const", bufs=1) as cp,
            tc.tile_pool(name="work", bufs=2) as wp,
            tc.tile_pool(name="dram", bufs=1, space="DRAM") as dr,
            tc.tile_pool(name="psum", bufs=2, space="PSUM") as pp_pre,
            tc.tile_pool(name="psum_t", bufs=2, space="PSUM") as pp_t,
            tc.tile_pool(name="psum_w", bufs=1, space="PSUM") as pp_w,
            tc.tile_pool(name="psum_wb", bufs=1, space="PSUM") as pp_wb,
        ):
            bnd_dram = dr.tile([64, S], bf16, tag="bnd_dram")
            delta_dram = dr.tile([P, NW * CAP_N], f32, tag="delta_dram")

            # ------- persistent SBUF tensors
            table = cp.tile([P, NV], f32, tag="table")
            idx12w = cp.tile([P, S // 16], i16, tag="idx12w")
            vacc = cp.tile([P, NW + 1], f32, tag="vacc")

            def cload(prm, shape, dt, tag):
                t = cp.tile(list(shape), dt, tag=tag)
                nc.sync.dma_start(out=t[:], in_=prm[:])
                return t

            nc.sync.dma_start(out=idx12w[:], in_=idx12_p[:])
            lhs_site = cload(lhs_site_p, (1, P), f32, "lhs_site")
            bias_site = cload(bias_site_p, (P, 1), f32, "bias_site")
            lhs_brep = cload(lhs_brep_p, (NGROUP, 80), f32, "lhs_brep")
            bias_cent = cload(bias_cent_p, (80, 1), f32, "bias_cent")
            lhs_wbond = cload(lhs_wbond_p, (80, 64), bf16, "lhs_wbond")
            bias_bond = cload(bias_bond_p, (64, 1), f32, "bias_bond")
            ident64 = cload(ident_p, (64, 64), bf16, "ident64")
            lhs_rep = cload(lhs_rep_p, (P, P), f32, "lhs_rep")
            bias_sig = cload(bias_sig_p, (64, N_BLOCKS), f32, "bias_sig")
            bias_sm = cload(bias_sm_p, (64, N_BLOCKS), f32, "bias_sm")
            blk_c = []
            for i in range(N_BLOCKS):
                blk_c.append((
                    cload(dp[f"lhsT_s2_{i}"], (P, P), bf16, f"lhs_s2_{i}"),
                    cload(dp[f"lhsT_bnd_{i}"], (64, P), bf16, f"lhs_bnd_{i}"),
                    cload(dp[f"WsigR_{i}"], (EMB, 64), f32, f"wsigr_{i}"),
                    cload(dp[f"WsmR_{i}"], (EMB, 64), f32, f"wsmr_{i}"),
                ))
            maskbd = cload(maskbd_p, (P, P), f32, "maskbd")
            t2wbd = cp.tile([P, NB * P], bf16, tag="t2wbd")
            nc.vector.memset(t2wbd[:], 0.0)
            w1 = cload(w1_p, (EMB, H1), f32, "w1")
            b1 = cload(b1_p, (H1, 1), f32, "b1")
            w2 = cload(w2_p, (H1, H2), f32, "w2")
            b2 = cload(b2_p, (H2, 1), f32, "b2")
            lhs_fold = cload(fold_p, (P, EMB), f32, "lhs_fold")
            lhs_id8 = cload(id8_p, (P, EMB), f32, "lhs_id8")
            w3 = cload(w3_p, (H2, 1), f32, "w3")
            b3 = cload(b3_p, (1, 1), f32, "b3")

            # ------- phase A1: table init
            ACOLS = 512
            wpa_cm = tc.tile_pool(name="worka", bufs=2)
            wpa = wpa_cm.__enter__()
            for j in range((NV + ACOLS - 1) // ACOLS):
                c0 = j * ACOLS
                c1 = min(NV, c0 + ACOLS)
                w = c1 - c0
                st = wpa.tile([1, ACOLS], f32, tag="sites")
                nc.sync.dma_start(out=st[:, :w], in_=sites_p[:, c0:c1])
                ps = pp_t.tile([P, 512], f32, tag="pt")
                nc.tensor.matmul(out=ps[:, :w], lhsT=lhs_site[:],
                                 rhs=st[:, :w], start=True, stop=True)
                nc.scalar.activation(table[:, c0:c1], ps[:, :w], AF.Identity,
                                     bias=bias_site[:])

            # ------- phase A2: bnd embedding build -> DRAM
            for j in range(S // ACOLS):
                c0 = j * ACOLS
                bt = wpa.tile([NGROUP, ACOLS], f32, tag="bondsin")
                nc.sync.dma_start(out=bt[:], in_=bonds_p[:, c0:c0 + ACOLS])
                ps = pp_t.tile([P, 512], f32, tag="pt")
                nc.tensor.matmul(out=ps[:80, :], lhsT=lhs_brep[:],
                                 rhs=bt[:], start=True, stop=True)
                gsq = wpa.tile([80, ACOLS], f32, tag="gsq")
                nc.scalar.activation(gsq[:], ps[:80, :], AF.Square,
                                     bias=bias_cent[:])
                gb = wpa.tile([80, ACOLS], bf16, tag="gb")
                nc.scalar.activation(gb[:], gsq[:], AF.Exp, scale=-1.0)
                bt2 = wpa.tile([64, ACOLS], bf16, tag="bnd64")
                ps2 = pp_t.tile([P, 512], f32, tag="pt")
                nc.tensor.matmul(out=ps2[:64, :], lhsT=lhs_wbond[:],
                                 rhs=gb[:], start=True, stop=True)
                nc.scalar.activation(bt2[:], ps2[:64, :], AF.Identity,
                                     bias=bias_bond[:])
                nc.sync.dma_start(out=bnd_dram[:, c0:c0 + ACOLS], in_=bt2[:])

            wpa_cm.__exit__(None, None, None)
            nc.vector.memset(vacc[:], 0.0)
            zerosP = cp.tile([P, P], bf16, tag="zerosP")
            zeros128 = cp.tile([P, CAP_N], bf16, tag="zeros128")
            nc.vector.memset(zerosP[:], 0.0)
            nc.vector.memset(zeros128[:], 0.0)

            # ------- phase B: conv blocks (software-pipelined: the compute
            # stage of window w overlaps the scatter stage of window w-1,
            # and the scatter accumulates into two separate PSUM banks to
            # break the per-bank read-modify-write chain)
            for i in range(N_BLOCKS):
                last = i == N_BLOCKS - 1
                l_s2, l_bnd, wsigr, wsmr = blk_c[i]

                # rebuild block-diagonal T2W slabs for this block's s1 path
                for bq in range((NB + 3) // 4):
                    psT = pp_t.tile([P, 512], f32, tag="pt")
                    nq = min(4, NB - 4 * bq)
                    for jj in range(nq):
                        j = 4 * bq + jj
                        cw = P
                        nc.tensor.matmul(
                            out=psT[:cw, 128 * jj : 128 * jj + 64],
                            lhsT=table[0:EMB, P * j : P * j + cw],
                            rhs=wsigr[:], start=True, stop=True)
                        nc.tensor.matmul(
                            out=psT[:cw, 128 * jj + 64 : 128 * jj + 128],
                            lhsT=table[0:EMB, P * j : P * j + cw],
                            rhs=wsmr[:], start=True, stop=True)
                    for jj in range(nq):
                        j = 4 * bq + jj
                        nc.vector.tensor_tensor(
                            out=t2wbd[:, P * j : P * j + P],
                            in0=psT[:, 128 * jj : 128 * jj + 128],
                            in1=maskbd[:], op=ALU.mult)

                state = {}

                def compute_stage(w):
                    c0 = w * TCOLS
                    s2g = wp.tile([P, TCOLS], f32, tag="s2g")
                    nc.gpsimd.ap_gather(
                        s2g[:], table[:],
                        idx12w[:, w * 64 : (w + 1) * 64],
                        channels=P, num_elems=NV, d=1, num_idxs=TCOLS)
                    s2gb = wp.tile([P, TCOLS], bf16, tag="s2gb")
                    nc.scalar.activation(s2gb[:], s2g[:], AF.Copy)
                    bnd_t = wp.tile([64, TCOLS], bf16, tag="bnd_t")
                    nc.sync.dma_start(out=bnd_t[:],
                                      in_=bnd_dram[:, c0:c0 + TCOLS])
                    ohw = wp.tile([P, TCOLS], bf16, tag="ohw")
                    nc.sync.dma_start(out=ohw[:],
                                      in_=ohexp_p[:, c0:c0 + TCOLS])
                    ps = pp_pre.tile([P, TCOLS], f32, tag="pre")
                    for h in range(2):
                        hs = slice(512 * h, 512 * h + 512)
                        nc.tensor.matmul(out=ps[:, hs], lhsT=l_s2[:],
                                         rhs=s2gb[:, hs], start=True,
                                         stop=False)
                        nc.tensor.matmul(out=ps[:, hs], lhsT=l_bnd[:],
                                         rhs=bnd_t[:, hs], start=False,
                                         stop=False)
                        for (sc0, sc1, j) in pp["segs"][w]:
                            a = max(sc0, 512 * h)
                            b2 = min(sc1, 512 * h + 512)
                            if a >= b2:
                                continue
                            nc.tensor.matmul(
                                out=ps[:, a:b2],
                                lhsT=t2wbd[:, P * j : P * j + P],
                                rhs=ohw[:, a:b2],
                                start=False, stop=False,
                                skip_group_check=True)
                        nc.tensor.matmul(out=ps[:, hs], lhsT=zerosP[:],
                                         rhs=s2gb[:, hs], start=False,
                                         stop=True, skip_group_check=True)
                    sig = wp.tile([64, TCOLS], bf16, tag="sig")
                    nc.scalar.activation(sig[:], ps[0:64, :], AF.Sigmoid,
                                         bias=bias_sig[:, i : i + 1])
                    v = wp.tile([64, TCOLS], bf16, tag="v")
                    nc.vector.tensor_scalar(
                        out=v[:], in0=ps[64:128, :],
                        scalar1=bias_sm[:, i : i + 1], scalar2=0.0,
                        op0=ALU.add, op1=ALU.max)
                    nc.vector.tensor_mul(v[:], v[:], sig[:])
                    vts = {}
                    for kk in range(2):
                        pst = pp_t.tile([P, 512], bf16, tag="pt")
                        for k4 in range(4):
                            k = 4 * kk + k4
                            nc.tensor.matmul(
                                out=pst[:, 64 * k4 : 64 * k4 + 64],
                                lhsT=v[:, k * P : k * P + P], rhs=ident64[:],
                                is_transpose=True, start=(k4 == 0),
                                stop=(k4 == 3))
                        vt = wp.tile([P, 256], bf16, tag="vt")
                        nc.scalar.activation(vt[:], pst[:, :256], AF.Copy)
                        vts[kk] = vt
                    state[w] = vts

                def scatter_stage(w):
                    vts = state.pop(w)
                    n0 = int(pp["win_n0"][w])
                    n1 = int(pp["win_n1"][w])
                    nw = n1 - n0
                    o0 = int(pp["oh_woff"][w])
                    o1 = int(pp["oh_woff"][w + 1])
                    ohsc = wp.tile([P, MAXW], bf16, tag="ohsc")
                    nc.sync.dma_start(out=ohsc[:, : o1 - o0],
                                      in_=oneh_p[:, o0:o1])
                    psw = pp_w.tile([P, CAP_N], f32, tag="win")
                    nc.tensor.matmul(
                        out=psw[:, 0:nw], lhsT=zerosP[:],
                        rhs=zeros128[:, 0:nw], start=True, stop=False,
                        skip_group_check=True)
                    nch = len(pp["chunks"][w])
                    for ci, (k, g, off, wdt, cc0, start) in \
                            enumerate(pp["chunks"][w]):
                        vt = vts[k // 4]
                        z = 32 * (ci % 3)
                        nc.tensor.matmul(
                            out=psw[z : z + EMB, cc0 : cc0 + wdt],
                            lhsT=vt[:, 64 * (k % 4) + 8 * g :
                                    64 * (k % 4) + 8 * g + EMB],
                            rhs=ohsc[:, off - o0 : off - o0 + wdt],
                            start=False, stop=(ci == nch - 1),
                            skip_group_check=True)
                    if last:
                        nc.vector.tensor_reduce(
                            vacc[:, w : w + 1], psw[:, :nw],
                            axis=mybir.AxisListType.X, op=ALU.add)
                    else:
                        d8o = wp.tile([P, CAP_N], f32, tag="d8o")
                        nc.scalar.activation(d8o[:, :nw], psw[:, :nw],
                                             AF.Copy)
                        nc.sync.dma_start(
                            out=delta_dram[:, w * CAP_N : w * CAP_N + nw],
                            in_=d8o[:, :nw])

                for w in range(NW + 1):
                    if w < NW:
                        compute_stage(w)
                    if w >= 1:
                        scatter_stage(w - 1)
                if not last:
                    for w in range(NW):
                        n0 = int(pp["win_n0"][w])
                        n1 = int(pp["win_n1"][w])
                        nw = n1 - n0
                        d8 = wp.tile([P, CAP_N], f32, tag="d8")
                        nc.sync.dma_start(
                            out=d8[:, :nw],
                            in_=delta_dram[:, w * CAP_N : w * CAP_N + nw])
                        psr = pp_t.tile([P, 512], f32, tag="pt")
                        nc.tensor.matmul(out=psr[:, :nw], lhsT=lhs_rep[:],
                                         rhs=d8[:, :nw], start=True,
                                         stop=True)
                        nc.vector.tensor_add(table[:, n0:n1],
                                             table[:, n0:n1], psr[:, :nw])

            # ------- phase C: mean + MLP
            tred = cp.tile([P, 2], f32, tag="tred")
            nc.vector.tensor_reduce(tred[0:EMB, 0:1], table[0:EMB, :],
                                    axis=mybir.AxisListType.X, op=ALU.add)
            nc.vector.tensor_reduce(tred[:, 1:2], vacc[:, :NW],
                                    axis=mybir.AxisListType.X, op=ALU.add)
            # fold the three staggered row-sets + table sum on PE, then
            # scale by 1/N on the way out of PSUM
            psv = pp_t.tile([P, 512], f32, tag="pt")
            nc.tensor.matmul(out=psv[:EMB, 0:1], lhsT=lhs_fold[:],
                             rhs=tred[:, 1:2], start=True, stop=False)
            nc.tensor.matmul(out=psv[:EMB, 0:1], lhsT=lhs_id8[:],
                             rhs=tred[:, 0:1], start=False, stop=True)
            vec = cp.tile([EMB, 1], f32, tag="vec")
            nc.scalar.activation(vec[:], psv[:EMB, 0:1], AF.Identity,
                                 scale=1.0 / N)
            psm = pp_t.tile([P, 512], f32, tag="pt")
            nc.tensor.matmul(out=psm[:H1, 0:1], lhsT=w1[:], rhs=vec[:],
                             start=True, stop=True)
            h1t = cp.tile([H1, 1], f32, tag="h1")
            nc.scalar.activation(h1t[:], psm[:H1, 0:1], AF.Relu, bias=b1[:])
            psm2 = pp_t.tile([P, 512], f32, tag="pt")
            nc.tensor.matmul(out=psm2[:H2, 0:1], lhsT=w2[:], rhs=h1t[:],
                             start=True, stop=True)
            h2t = cp.tile([H2, 1], f32, tag="h2")
            nc.scalar.activation(h2t[:], psm2[:H2, 0:1], AF.Relu, bias=b2[:])
            psm3 = pp_t.tile([P, 512], f32, tag="pt")
            nc.tensor.matmul(out=psm3[:1, 0:1], lhsT=w3[:], rhs=h2t[:],
                             start=True, stop=True)
            ot = cp.tile([1, 1], f32, tag="ot")
            nc.scalar.activation(ot[:], psm3[:1, 0:1], AF.Identity,
                                 bias=b3[:])
            nc.sync.dma_start(out=out_p[:], in_=ot[:])

    nc.compile()
    return nc


def _in_maps(pp, cn, sites, bonds, mlp):
    import ml_dtypes
    bf16 = ml_dtypes.bfloat16
    shared = {
        "idx12w": pp["idx12w"],
        "oneh": pp["oneh"].astype(bf16),
        "oh_exp": pp["oh_exp"].astype(bf16),
    }
    for k, v in cn.items():
        if (k == "ident64" or k == "lhsT_wbond" or k.startswith("lhsT_bnd_")
                or k.startswith("lhsT_s2_")):
            shared[k] = v.astype(bf16)
        else:
            shared[k] = v.astype(F32)
    shared.update(mlp)
    in_maps = []
    esrc = pp["esrc"]
    inv = np.full(pp["NV"], N, np.int64)
    inv[pp["newid"]] = np.arange(N)
    for b in range(B):
        bsorted = bonds[b, :, 0][pp["order"]]
        bg = np.where(pp["pad"], 0.0, bsorted[np.clip(esrc, 0, None)])
        m = dict(shared)
        sp = np.concatenate([sites[b, :, 0], [0.0]]).astype(F32)
        m["sites"] = np.ascontiguousarray(sp[inv][None, :], F32)
        m["bonds_g"] = bg.astype(F32)
        in_maps.append(m)
    return in_maps


def kernel(sites, bonds, idx1, idx2, W_site, b_site, W_bond, b_bond,
           W_sig, b_sig, W_sm, b_sm, W1, b1, W2, b2, W3, b3):
    sites = np.asarray(sites, F32)
    bonds = np.asarray(bonds, F32)
    pp = _prep(np.asarray(idx1), np.asarray(idx2))
    cn = _consts(pp, np.asarray(W_site, F32), np.asarray(b_site, F32),
                 np.asarray(W_bond, F32), np.asarray(b_bond, F32),
                 np.asarray(W_sig, F32), np.asarray(b_sig, F32),
                 np.asarray(W_sm, F32), np.asarray(b_sm, F32))
    mlp = {
        "W1": np.asarray(W1, F32), "b1": np.asarray(b1, F32)[:, None],
        "W2": np.asarray(W2, F32), "b2": np.asarray(b2, F32)[:, None],
        "W3": np.asarray(W3, F32), "b3": np.asarray(b3, F32)[:, None],
    }
    nc = _build_bass(pp)
    in_maps = _in_maps(pp, cn, sites, bonds, mlp)
    from concourse.bass_utils import run_bass_kernel_spmd
    res = run_bass_kernel_spmd(nc, in_maps, list(range(B)))
    global LAST_RESULT
    LAST_RESULT = res
    out = np.stack([np.asarray(res.results[b]["out"]).reshape(1)
                    for b in range(B)], axis=0)
    return out.astype(F32)


LAST_RESULT = None



# revision 21
# speedup vs baseline: 1.7367x; 1.1498x over previous
"""CGCNN message-passing kernel for 8 Trainium2 NeuronCores.

Strategy: data-parallel over the batch (structure b -> core b). The graph
(idx1/idx2) is shared across the batch and known at build time, so all
gather/scatter bookkeeping is precomputed on the host and baked into the
kernel as static access patterns + small data tensors.

Per-core device algorithm (N=20000 nodes, E=320000 edges, EMB=8):
  - node table kept in SBUF as [128, N] f32: row 16g+r holds s[:, r] for
    r<8 (replicated for the 8 GPSIMD cores), rows 16g+8.. are junk.
  - edges sorted by idx1, grouped into "windows" (<=512 nodes, <=8192
    edges), each window's edges split into 8 groups of <=1024 (padded).
  - per block: gpsimd.ap_gather pulls s[idx1], s[idx2] into [128, 1024]
    tiles (group g's stream on partitions 16g..16g+15); three fused
    block-diagonal matmuls (s1, s2, gaussian-bond) produce sigmoid and
    softmax-branch pre-activations in PSUM [128, 1024]; ACT sigmoid(+bias)
    and DVE relu(+bias) and DVE mul give v in bf16; PE transposes flip
    edges onto partitions; per-128-edge matmuls with static one-hot rhs
    accumulate node deltas into a PSUM window; deltas go to DRAM and are
    applied to the table at block end (skipped for the last block, where
    only the edge-sum is needed for the mean).
"""

import numpy as np

# ---------------------------------------------------------------- constants
B, N, E = 8, 20000, 320000
EMB = 8
CENTERS = 10
H1 = H2 = 24
N_BLOCKS = 6
MX_D, MN_D, WIDTH = 10.0, 0.0, 1.0
CAT = 3 * EMB  # 24

NGROUP = 8          # gpsimd cores per NeuronCore
TCOLS = 1024        # columns per group per window tile
CAP_E = NGROUP * TCOLS   # max edges per window
CAP_N = 512         # max nodes per window (one PSUM bank)
P = 128

F32 = np.float32
I16 = np.int16


# ---------------------------------------------------------------- host prep
def _prep(idx1: np.ndarray, idx2: np.ndarray, sim_safe: bool = False) -> dict:
    """Sort/partition the graph into windows, groups and scatter chunks.

    Nodes are renumbered block-major: new id = 128*j + 16*g + u, where the
    128-node block j is dealt snake-wise from the degree-sorted node list so
    every group's 16-node slice of a block has a near-equal edge count.
    Group g's stream holds edges whose (new) dest is in rows [16g,16g+16)
    of a block; a window is 1-4 consecutive blocks.
    """
    idx1 = np.asarray(idx1, np.int64)
    idx2 = np.asarray(idx2, np.int64)

    deg = np.bincount(idx1, minlength=N)
    byd = np.argsort(-deg, kind="stable")
    NB = (N + P - 1) // P
    newid = np.zeros(N, np.int64)
    for j in range(NB):
        blk = byd[j * P : (j + 1) * P]
        for k, old in enumerate(blk):
            newid[old] = P * j + 16 * (k % NGROUP) + (k // NGROUP)
    idx1 = newid[idx1]
    idx2 = newid[idx2]

    order = np.argsort(idx1, kind="stable")
    i1s = idx1[order]
    i2s = idx2[order]

    counts = np.bincount(i1s, minlength=NB * P)
    edge_start = np.concatenate([[0], np.cumsum(counts)])
    # per (block, group) edge count and block width = max over groups
    cnt_bg = counts.reshape(NB, NGROUP, 16).sum(axis=2)
    width_b = cnt_bg.max(axis=1)

    # windows: consecutive blocks, <=4 blocks, <=TCOLS columns
    win_b0, win_b1 = [], []
    j = 0
    while j < NB:
        b0 = j
        wcols = 0
        while j < NB and (j - b0) < 4 and wcols + width_b[j] <= TCOLS:
            wcols += width_b[j]
            j += 1
        if j == b0:
            raise RuntimeError("block exceeds window capacity")
        win_b0.append(b0)
        win_b1.append(j)
    NW = len(win_b0)
    win_n0 = [b0 * P for b0 in win_b0]
    win_n1 = [b1 * P for b1 in win_b1]

    S = NW * TCOLS  # columns per group

    # esrc[g, c] + per-window block column ranges (segments)
    esrc = np.full((NGROUP, S), -1, np.int64)
    segs = []  # per window: list of (c0, c1, block j)
    for w in range(NW):
        col = 0
        wsegs = []
        for j in range(win_b0[w], win_b1[w]):
            wd = int(width_b[j])
            if wd > 0:
                wsegs.append((col, col + wd, j))
            for g in range(NGROUP):
                e0 = int(edge_start[P * j + 16 * g])
                ln = int(cnt_bg[j, g])
                esrc[g, w * TCOLS + col : w * TCOLS + col + ln] = \
                    np.arange(e0, e0 + ln)
            col += wd
        segs.append(wsegs)
    pad = esrc < 0

    def streams(vals_sorted):
        return np.where(pad, 0, vals_sorted[np.clip(esrc, 0, None)])

    g1 = streams(i1s)  # [NGROUP, S] destination node per column
    g2 = streams(i2s)

    def wrap(stream):
        # ap_gather layout: index i of core-g stream at [16g + i%16, i//16]
        out = np.zeros((P, S // 16), I16)
        for g in range(NGROUP):
            out[16 * g : 16 * g + 16, :] = stream[g].reshape(S // 16, 16).T
        return out

    idx1w = wrap(g1)
    idx2w = wrap(g2)
    # s2-only per-window stream (s1 is produced by expansion matmuls)
    comb = np.zeros((P, S // 16), I16)
    for w in range(NW):
        cs = slice(w * TCOLS, (w + 1) * TCOLS)
        for g in range(NGROUP):
            comb[16 * g : 16 * g + 16, w * 64 : (w + 1) * 64] = \
                g2[g, cs].reshape(TCOLS // 16, 16).T

    # ---- s1 expansion 8-hot rhs [128, NW*TCOLS]: column (w*1024+c) has a
    # one at row (dest & 127) for each group with a real edge there (eight
    # distinct rows since dest = 128j + 16g + u).
    oh_cols_exp = np.zeros((P, NW * TCOLS), np.float32)
    for g in range(NGROUP):
        real = ~pad[g]
        cols = np.nonzero(real)[0]
        oh_cols_exp[g1[g, cols] & (P - 1), cols] = 1.0

    # ---- scatter chunks: (window w, col-range k, group g) of 128 edges
    oh_cols = []           # list of [128, width] float arrays
    chunks = []            # per window: list of (k, g, off, wdt, c0, start)
    oh_total = 0
    for w in range(NW):
        n0, n1 = win_n0[w], win_n1[w]
        nw = n1 - n0
        covered = np.zeros(nw, bool)
        wchunks = []
        first = True
        for k in range(TCOLS // P):
            for g in range(NGROUP):
                cols = slice(w * TCOLS + k * P, w * TCOLS + k * P + P)
                e = esrc[g, cols]
                real = e >= 0
                if not real.any() and not first:
                    continue
                if real.any():
                    loc = g1[g, cols] - n0
                    c0, c1 = int(loc[real].min()), int(loc[real].max()) + 1
                else:
                    loc = np.zeros(P, np.int64)
                    c0, c1 = 0, 1
                oh = np.zeros((P, c1 - c0), F32)
                oh[np.nonzero(real)[0], loc[real] - c0] = 1.0
                oh_cols.append(oh)
                if first:
                    wchunks.append([k, g, oh_total, c1 - c0, c0, True])
                elif not sim_safe:
                    wchunks.append([k, g, oh_total, c1 - c0, c0, False])
                else:
                    # split at written/fresh boundaries so each matmul region
                    # is uniformly accumulated or overwritten (PSUM
                    # has_written is per element; the sim wants uniformity)
                    a = c0
                    while a < c1:
                        st8 = bool(covered[a])
                        b = a
                        while b < c1 and bool(covered[b]) == st8:
                            b += 1
                        wchunks.append([k, g, oh_total + (a - c0), b - a,
                                        a, False])
                        a = b
                covered[c0:c1] = True
                oh_total += c1 - c0
                first = False
        # never-written columns (deg-0 nodes outside all chunk spans):
        # emit zero one-hot chunks per contiguous run so the window PSUM
        # is fully initialized before it is read.
        miss = np.nonzero(~covered)[0]
        if len(miss):
            runs = np.split(miss, np.nonzero(np.diff(miss) != 1)[0] + 1)
            for run in runs:
                oh_cols.append(np.zeros((P, len(run)), F32))
                wchunks.append([0, 0, oh_total, len(run), int(run[0]), False])
                oh_total += len(run)
        chunks.append(wchunks)

    oneh = np.concatenate(oh_cols, axis=1) if oh_cols else np.zeros((P, 0), F32)

    oh_woff = [wch[0][2] for wch in chunks] + [oneh.shape[1]]
    wcols = []
    for w in range(NW):
        tot = sum(int(width_b[j]) for j in range(win_b0[w], win_b1[w]))
        wcols.append(tot)
    return dict(
        order=order, i1s=i1s, i2s=i2s, esrc=esrc, pad=pad,
        win_n0=np.array(win_n0), win_n1=np.array(win_n1), NW=NW, S=S,
        idx1w=idx1w, idx2w=idx2w, idx12w=comb, oneh=oneh,
        chunks=chunks, oh_exp=oh_cols_exp, segs=segs, NB=NB,
        newid=newid, oh_woff=oh_woff, NV=NB * P, wcols=wcols,
    )


# ------------------------------------------------------- host-built weights
def _consts(pp, W_site, b_site, W_bond, b_bond, W_sig, b_sig, W_sm, b_sm):
    """Build all static lhsT / bias tensors in device layouts."""
    c = {}
    # table init: psum[16g+r, :] = W_site[0, r] * sites
    t = np.zeros((1, P), F32)
    for g in range(NGROUP):
        t[0, 16 * g : 16 * g + EMB] = W_site[0]
    c["lhsT_site"] = t
    bias = np.zeros((P, 1), F32)
    for g in range(NGROUP):
        bias[16 * g : 16 * g + EMB, 0] = b_site
    c["bias_site"] = bias

    # bonds replicate [8 -> 80]
    t = np.zeros((NGROUP, 80), F32)
    for g in range(NGROUP):
        t[g, 10 * g : 10 * g + CENTERS] = 1.0
    c["lhsT_brep"] = t
    cent = np.linspace(MN_D, MX_D, CENTERS, dtype=F32)
    bias = np.zeros((80, 1), F32)
    for g in range(NGROUP):
        bias[10 * g : 10 * g + CENTERS, 0] = -cent
    c["bias_cent"] = bias

    # gaussian -> bond embedding: [80, 64]
    t = np.zeros((80, 64), F32)
    for g in range(NGROUP):
        t[10 * g : 10 * g + CENTERS, 8 * g : 8 * g + EMB] = W_bond
    c["lhsT_wbond"] = t
    bias = np.zeros((64, 1), F32)
    for g in range(NGROUP):
        bias[8 * g : 8 * g + EMB, 0] = b_bond
    c["bias_bond"] = bias

    # per-block lhsT for s2 [128, 128], bnd [64, 128], biases [64, NB];
    # s1 is folded into T2W (tableT @ W) rebuilt on-device per block
    bs_sig = np.zeros((64, N_BLOCKS), F32)
    bs_sm = np.zeros((64, N_BLOCKS), F32)
    for i in range(N_BLOCKS):
        for nm, rows in (("s2", slice(8, 16)),):
            t = np.zeros((P, P), F32)
            for g in range(NGROUP):
                t[16 * g : 16 * g + EMB, 8 * g : 8 * g + EMB] = W_sig[i][rows]
                t[16 * g : 16 * g + EMB, 64 + 8 * g : 64 + 8 * g + EMB] = \
                    W_sm[i][rows]
            c[f"lhsT_{nm}_{i}"] = t
        c[f"WsigR_{i}"] = np.tile(np.ascontiguousarray(W_sig[i][0:EMB], F32),
                                  (1, NGROUP))
        c[f"WsmR_{i}"] = np.tile(np.ascontiguousarray(W_sm[i][0:EMB], F32),
                                 (1, NGROUP))
        t = np.zeros((64, P), F32)
        for g in range(NGROUP):
            t[8 * g : 8 * g + EMB, 8 * g : 8 * g + EMB] = W_sig[i][16:24]
            t[8 * g : 8 * g + EMB, 64 + 8 * g : 64 + 8 * g + EMB] = W_sm[i][16:24]
        c[f"lhsT_bnd_{i}"] = t
        for g in range(NGROUP):
            bs_sig[8 * g : 8 * g + EMB, i] = b_sig[i]
            bs_sm[8 * g : 8 * g + EMB, i] = b_sm[i]
    c["bias_sig"] = bs_sig
    c["bias_sm"] = bs_sm

    mbd = np.zeros((P, P), F32)
    for g in range(NGROUP):
        for u in range(16):
            mbd[16 * g + u, 8 * g : 8 * g + 8] = 1.0
            mbd[16 * g + u, 64 + 8 * g : 64 + 8 * g + 8] = 1.0
    c["maskBD"] = mbd
    # transpose identity [64, 64] and replicate matrix [8, 128]
    c["ident64"] = np.eye(64, dtype=F32)
    t = np.zeros((P, P), F32)
    for z in range(3):
        for g in range(NGROUP):
            t[32 * z : 32 * z + EMB, 16 * g : 16 * g + EMB] = \
                np.eye(EMB, dtype=F32)
    c["lhsT_rep"] = t
    t = np.zeros((P, EMB), F32)
    for z in range(3):
        t[32 * z : 32 * z + EMB, :] = np.eye(EMB, dtype=F32)
    c["lhsT_fold"] = t
    t = np.zeros((P, EMB), F32)
    t[0:EMB, :] = np.eye(EMB, dtype=F32)
    c["lhsT_id8"] = t
    return c


# ---------------------------------------------------- numpy model (testing)
def _numpy_model(pp, cn, sites, bonds_g, W1, b1, W2, b2, W3, b3):
    """Exact simulation of the device algorithm (one structure)."""
    import ml_dtypes
    bf16 = ml_dtypes.bfloat16

    def tobf(x):
        return x.astype(bf16).astype(F32)

    S, NW = pp["S"], pp["NW"]
    table = (cn["lhsT_site"].T @ sites[None, :]) + cn["bias_site"]
    brep = cn["lhsT_brep"].T @ bonds_g                      # [80, S]
    gsq = np.square(brep + cn["bias_cent"])
    gb = tobf(np.exp(-gsq))
    bnd64 = tobf(cn["lhsT_wbond"].T @ gb + cn["bias_bond"])  # [64, S]

    def unwrap(w):
        out = np.zeros((NGROUP, S), np.int64)
        for g in range(NGROUP):
            out[g] = w[16 * g : 16 * g + 16, :].T.reshape(-1)
        return out

    st1 = unwrap(pp["idx1w"])
    st2 = unwrap(pp["idx2w"])

    vacc = np.zeros(EMB, F32)
    for i in range(N_BLOCKS):
        # T2W: bf16(tableT @ W) per node, s1 contribution via one-hot
        t2wsig = tobf(table[0:EMB].T @ cn[f"WsigR_{i}"][:, 0:EMB])  # [NV, 8]
        t2wsm = tobf(table[0:EMB].T @ cn[f"WsmR_{i}"][:, 0:EMB])
        delta = np.zeros((EMB, table.shape[1]), F32)
        for w in range(NW):
            cols = slice(w * TCOLS, (w + 1) * TCOLS)
            s2g = np.zeros((P, TCOLS), F32)
            for g in range(NGROUP):
                rows = slice(16 * g, 16 * g + 16)
                s2g[rows] = table[rows][:, st2[g, cols]]
            pre = (
                tobf(cn[f"lhsT_s2_{i}"]).T @ tobf(s2g)
                + cn[f"lhsT_bnd_{i}"].T @ bnd64[:, cols]
            )
            wpad = pp["pad"][:, cols]
            for g in range(NGROUP):
                sel = ~wpad[g]
                n1w = st1[g, cols][sel]
                pre[8 * g : 8 * g + EMB, sel] += t2wsig[n1w].T
                pre[64 + 8 * g : 64 + 8 * g + EMB, sel] += t2wsm[n1w].T
            sig = tobf(1.0 / (1.0 + np.exp(-(pre[0:64] + cn["bias_sig"][:, i:i+1]))))
            sm = tobf(np.maximum(pre[64:128] + cn["bias_sm"][:, i:i+1], 0.0))
            v = tobf(sig * sm)                               # [64, TCOLS]
            n0, n1 = pp["win_n0"][w], pp["win_n1"][w]
            psw = np.zeros((EMB, CAP_N), F32)
            for (k, g, off, wdt, c0, start) in pp["chunks"][w]:
                vt = v[8 * g : 8 * g + EMB, k * P : k * P + P]   # [8, 128]
                oh = pp["oneh_f32"][:, off : off + wdt]
                contrib = vt @ oh                                 # [8, wdt]
                if start:
                    psw[:] = 0.0
                psw[:, c0 : c0 + wdt] += contrib
            nw = n1 - n0
            if i == N_BLOCKS - 1:
                vacc += psw[:, :nw].sum(axis=1)
            else:
                delta[:, n0:n1] = psw[:, :nw]
        if i < N_BLOCKS - 1:
            table = table + cn["lhsT_rep"][0:EMB].T @ delta

    tred = table[0:EMB].sum(axis=1)
    vec = (tred + vacc) / N
    h = np.maximum(vec @ W1 + b1, 0.0)
    h = np.maximum(h @ W2 + b2, 0.0)
    return h @ W3 + b3


# ------------------------------------------------------------- bass kernel
def _build_bass(pp):
    import concourse.bass as bass
    import concourse.bacc as bacc
    import concourse.mybir as mybir
    from concourse.tile import TileContext

    AF = mybir.ActivationFunctionType
    ALU = mybir.AluOpType
    f32, bf16, i16 = mybir.dt.float32, mybir.dt.bfloat16, mybir.dt.int16

    S, NW = pp["S"], pp["NW"]
    OH = pp["oneh"].shape[1]
    MAXW = max(pp["oh_woff"][w + 1] - pp["oh_woff"][w] for w in range(NW))
    NT = S // TCOLS  # == NW column tiles per pass

    nc = bacc.Bacc(None, target_bir_lowering=False, debug=False)

    dp = {}

    def param(name, shape, dt):
        dp[name] = nc.declare_dram_parameter(name, list(shape), dt,
                                             isOutput=False)
        return dp[name]

    NB = pp["NB"]
    NV = pp["NV"]
    sites_p = param("sites", (1, NV), f32)
    bonds_p = param("bonds_g", (NGROUP, S), f32)
    idx12_p = param("idx12w", (P, S // 16), i16)
    oneh_p = param("oneh", (P, OH), bf16)
    ohexp_p = param("oh_exp", (P, NW * TCOLS), bf16)
    lhs_site_p = param("lhsT_site", (1, P), f32)
    bias_site_p = param("bias_site", (P, 1), f32)
    lhs_brep_p = param("lhsT_brep", (NGROUP, 80), f32)
    bias_cent_p = param("bias_cent", (80, 1), f32)
    lhs_wbond_p = param("lhsT_wbond", (80, 64), bf16)
    bias_bond_p = param("bias_bond", (64, 1), f32)
    for i in range(N_BLOCKS):
        param(f"lhsT_s2_{i}", (P, P), bf16)
        param(f"lhsT_bnd_{i}", (64, P), bf16)
        param(f"WsigR_{i}", (EMB, 64), f32)
        param(f"WsmR_{i}", (EMB, 64), f32)
    bias_sig_p = param("bias_sig", (64, N_BLOCKS), f32)
    bias_sm_p = param("bias_sm", (64, N_BLOCKS), f32)
    ident_p = param("ident64", (64, 64), bf16)
    maskbd_p = param("maskBD", (P, P), f32)
    lhs_rep_p = param("lhsT_rep", (P, P), f32)
    w1_p = param("W1", (EMB, H1), f32)
    b1_p = param("b1", (H1, 1), f32)
    w2_p = param("W2", (H1, H2), f32)
    b2_p = param("b2", (H2, 1), f32)
    fold_p = param("lhsT_fold", (P, EMB), f32)
    id8_p = param("lhsT_id8", (P, EMB), f32)
    w3_p = param("W3", (H2, 1), f32)
    b3_p = param("b3", (1, 1), f32)
    out_p = nc.declare_dram_parameter("out", [1, 1], f32, isOutput=True)

    with TileContext(nc) as tc:
        with (
            tc.tile_pool(name="const", bufs=1) as cp,
            tc.tile_pool(name="work", bufs=2) as wp,
            tc.tile_pool(name="dram", bufs=1, space="DRAM") as dr,
            tc.tile_pool(name="psum", bufs=2, space="PSUM") as pp_pre,
            tc.tile_pool(name="psum_t", bufs=2, space="PSUM") as pp_t,
            tc.tile_pool(name="psum_w", bufs=1, space="PSUM") as pp_w,
            tc.tile_pool(name="psum_wb", bufs=1, space="PSUM") as pp_wb,
        ):
            bnd_dram = dr.tile([64, S], bf16, tag="bnd_dram")
            delta_dram = dr.tile([P, NW * CAP_N], f32, tag="delta_dram")

            # ------- persistent SBUF tensors
            table = cp.tile([P, NV], f32, tag="table")
            idx12w = cp.tile([P, S // 16], i16, tag="idx12w")
            vacc = cp.tile([P, NW + 1], f32, tag="vacc")

            def cload(prm, shape, dt, tag):
                t = cp.tile(list(shape), dt, tag=tag)
                nc.sync.dma_start(out=t[:], in_=prm[:])
                return t

            nc.sync.dma_start(out=idx12w[:], in_=idx12_p[:])
            lhs_site = cload(lhs_site_p, (1, P), f32, "lhs_site")
            bias_site = cload(bias_site_p, (P, 1), f32, "bias_site")
            lhs_brep = cload(lhs_brep_p, (NGROUP, 80), f32, "lhs_brep")
            bias_cent = cload(bias_cent_p, (80, 1), f32, "bias_cent")
            lhs_wbond = cload(lhs_wbond_p, (80, 64), bf16, "lhs_wbond")
            bias_bond = cload(bias_bond_p, (64, 1), f32, "bias_bond")
            ident64 = cload(ident_p, (64, 64), bf16, "ident64")
            lhs_rep = cload(lhs_rep_p, (P, P), f32, "lhs_rep")
            bias_sig = cload(bias_sig_p, (64, N_BLOCKS), f32, "bias_sig")
            bias_sm = cload(bias_sm_p, (64, N_BLOCKS), f32, "bias_sm")
            blk_c = []
            for i in range(N_BLOCKS):
                blk_c.append((
                    cload(dp[f"lhsT_s2_{i}"], (P, P), bf16, f"lhs_s2_{i}"),
                    cload(dp[f"lhsT_bnd_{i}"], (64, P), bf16, f"lhs_bnd_{i}"),
                    cload(dp[f"WsigR_{i}"], (EMB, 64), f32, f"wsigr_{i}"),
                    cload(dp[f"WsmR_{i}"], (EMB, 64), f32, f"wsmr_{i}"),
                ))
            maskbd = cload(maskbd_p, (P, P), f32, "maskbd")
            t2wbd = cp.tile([P, NB * P], bf16, tag="t2wbd")
            nc.vector.memset(t2wbd[:], 0.0)
            w1 = cload(w1_p, (EMB, H1), f32, "w1")
            b1 = cload(b1_p, (H1, 1), f32, "b1")
            w2 = cload(w2_p, (H1, H2), f32, "w2")
            b2 = cload(b2_p, (H2, 1), f32, "b2")
            lhs_fold = cload(fold_p, (P, EMB), f32, "lhs_fold")
            lhs_id8 = cload(id8_p, (P, EMB), f32, "lhs_id8")
            w3 = cload(w3_p, (H2, 1), f32, "w3")
            b3 = cload(b3_p, (1, 1), f32, "b3")

            # ------- phase A1: table init
            ACOLS = 512
            wpa_cm = tc.tile_pool(name="worka", bufs=2)
            wpa = wpa_cm.__enter__()
            for j in range((NV + ACOLS - 1) // ACOLS):
                c0 = j * ACOLS
                c1 = min(NV, c0 + ACOLS)
                w = c1 - c0
                st = wpa.tile([1, ACOLS], f32, tag="sites")
                nc.sync.dma_start(out=st[:, :w], in_=sites_p[:, c0:c1])
                ps = pp_t.tile([P, 512], f32, tag="pt")
                nc.tensor.matmul(out=ps[:, :w], lhsT=lhs_site[:],
                                 rhs=st[:, :w], start=True, stop=True)
                nc.scalar.activation(table[:, c0:c1], ps[:, :w], AF.Identity,
                                     bias=bias_site[:])

            # ------- phase A2: bnd embedding build -> DRAM
            for j in range(S // ACOLS):
                c0 = j * ACOLS
                bt = wpa.tile([NGROUP, ACOLS], f32, tag="bondsin")
                nc.sync.dma_start(out=bt[:], in_=bonds_p[:, c0:c0 + ACOLS])
                ps = pp_t.tile([P, 512], f32, tag="pt")
                nc.tensor.matmul(out=ps[:80, :], lhsT=lhs_brep[:],
                                 rhs=bt[:], start=True, stop=True)
                gsq = wpa.tile([80, ACOLS], f32, tag="gsq")
                nc.scalar.activation(gsq[:], ps[:80, :], AF.Square,
                                     bias=bias_cent[:])
                gb = wpa.tile([80, ACOLS], bf16, tag="gb")
                nc.scalar.activation(gb[:], gsq[:], AF.Exp, scale=-1.0)
                bt2 = wpa.tile([64, ACOLS], bf16, tag="bnd64")
                ps2 = pp_t.tile([P, 512], f32, tag="pt")
                nc.tensor.matmul(out=ps2[:64, :], lhsT=lhs_wbond[:],
                                 rhs=gb[:], start=True, stop=True)
                nc.scalar.activation(bt2[:], ps2[:64, :], AF.Identity,
                                     bias=bias_bond[:])
                nc.sync.dma_start(out=bnd_dram[:, c0:c0 + ACOLS], in_=bt2[:])

            wpa_cm.__exit__(None, None, None)
            nc.vector.memset(vacc[:], 0.0)
            zerosP = cp.tile([P, P], bf16, tag="zerosP")
            zeros128 = cp.tile([P, CAP_N], bf16, tag="zeros128")
            nc.vector.memset(zerosP[:], 0.0)
            nc.vector.memset(zeros128[:], 0.0)

            # ------- phase B: conv blocks (software-pipelined: the compute
            # stage of window w overlaps the scatter stage of window w-1,
            # and the scatter accumulates into two separate PSUM banks to
            # break the per-bank read-modify-write chain)
            for i in range(N_BLOCKS):
                last = i == N_BLOCKS - 1
                l_s2, l_bnd, wsigr, wsmr = blk_c[i]

                # rebuild block-diagonal T2W slabs for this block's s1 path
                for bq in range((NB + 3) // 4):
                    psT = pp_t.tile([P, 512], f32, tag="pt")
                    nq = min(4, NB - 4 * bq)
                    for jj in range(nq):
                        j = 4 * bq + jj
                        cw = P
                        nc.tensor.matmul(
                            out=psT[:cw, 128 * jj : 128 * jj + 64],
                            lhsT=table[0:EMB, P * j : P * j + cw],
                            rhs=wsigr[:], start=True, stop=True)
                        nc.tensor.matmul(
                            out=psT[:cw, 128 * jj + 64 : 128 * jj + 128],
                            lhsT=table[0:EMB, P * j : P * j + cw],
                            rhs=wsmr[:], start=True, stop=True)
                    for jj in range(nq):
                        j = 4 * bq + jj
                        nc.vector.tensor_tensor(
                            out=t2wbd[:, P * j : P * j + P],
                            in0=psT[:, 128 * jj : 128 * jj + 128],
                            in1=maskbd[:], op=ALU.mult)

                state = {}

                def compute_stage(w):
                    c0 = w * TCOLS
                    wc16 = min(TCOLS, -(-int(pp["wcols"][w]) // 16) * 16)
                    s2g = wp.tile([P, TCOLS], f32, tag="s2g")
                    nc.gpsimd.ap_gather(
                        s2g[:, :wc16], table[:],
                        idx12w[:, w * 64 : w * 64 + wc16 // 16],
                        channels=P, num_elems=NV, d=1, num_idxs=wc16)
                    s2gb = wp.tile([P, TCOLS], bf16, tag="s2gb")
                    nc.scalar.activation(s2gb[:, :wc16], s2g[:, :wc16],
                                         AF.Copy)
                    if wc16 < TCOLS:
                        nc.vector.memset(s2gb[:, wc16:], 0.0)
                    bnd_t = wp.tile([64, TCOLS], bf16, tag="bnd_t")
                    nc.sync.dma_start(out=bnd_t[:],
                                      in_=bnd_dram[:, c0:c0 + TCOLS])
                    ohw = wp.tile([P, TCOLS], bf16, tag="ohw")
                    nc.sync.dma_start(out=ohw[:],
                                      in_=ohexp_p[:, c0:c0 + TCOLS])
                    ps = pp_pre.tile([P, TCOLS], f32, tag="pre")
                    for h in range(2):
                        hs = slice(512 * h, 512 * h + 512)
                        nc.tensor.matmul(out=ps[:, hs], lhsT=l_s2[:],
                                         rhs=s2gb[:, hs], start=True,
                                         stop=False)
                        nc.tensor.matmul(out=ps[:, hs], lhsT=l_bnd[:],
                                         rhs=bnd_t[:, hs], start=False,
                                         stop=False)
                        for (sc0, sc1, j) in pp["segs"][w]:
                            a = max(sc0, 512 * h)
                            b2 = min(sc1, 512 * h + 512)
                            if a >= b2:
                                continue
                            nc.tensor.matmul(
                                out=ps[:, a:b2],
                                lhsT=t2wbd[:, P * j : P * j + P],
                                rhs=ohw[:, a:b2],
                                start=False, stop=False,
                                skip_group_check=True)
                        nc.tensor.matmul(out=ps[:, hs], lhsT=zerosP[:],
                                         rhs=s2gb[:, hs], start=False,
                                         stop=True, skip_group_check=True)
                    sig = wp.tile([64, TCOLS], bf16, tag="sig")
                    nc.scalar.activation(sig[:], ps[0:64, :], AF.Sigmoid,
                                         bias=bias_sig[:, i : i + 1])
                    v = wp.tile([64, TCOLS], bf16, tag="v")
                    nc.vector.tensor_scalar(
                        out=v[:], in0=ps[64:128, :],
                        scalar1=bias_sm[:, i : i + 1], scalar2=0.0,
                        op0=ALU.add, op1=ALU.max)
                    nc.vector.tensor_mul(v[:], v[:], sig[:])
                    vts = {}
                    for kk in range(2):
                        pst = pp_t.tile([P, 512], bf16, tag="pt")
                        for k4 in range(4):
                            k = 4 * kk + k4
                            nc.tensor.matmul(
                                out=pst[:, 64 * k4 : 64 * k4 + 64],
                                lhsT=v[:, k * P : k * P + P], rhs=ident64[:],
                                is_transpose=True, start=(k4 == 0),
                                stop=(k4 == 3))
                        vt = wp.tile([P, 256], bf16, tag="vt")
                        nc.scalar.activation(vt[:], pst[:, :256], AF.Copy)
                        vts[kk] = vt
                    state[w] = vts

                def scatter_stage(w):
                    vts = state.pop(w)
                    n0 = int(pp["win_n0"][w])
                    n1 = int(pp["win_n1"][w])
                    nw = n1 - n0
                    o0 = int(pp["oh_woff"][w])
                    o1 = int(pp["oh_woff"][w + 1])
                    ohsc = wp.tile([P, MAXW], bf16, tag="ohsc")
                    nc.sync.dma_start(out=ohsc[:, : o1 - o0],
                                      in_=oneh_p[:, o0:o1])
                    psw = pp_w.tile([P, CAP_N], f32, tag="win")
                    nc.tensor.matmul(
                        out=psw[:, 0:nw], lhsT=zerosP[:],
                        rhs=zeros128[:, 0:nw], start=True, stop=False,
                        skip_group_check=True)
                    nch = len(pp["chunks"][w])
                    for ci, (k, g, off, wdt, cc0, start) in \
                            enumerate(pp["chunks"][w]):
                        vt = vts[k // 4]
                        z = 32 * (ci % 3)
                        nc.tensor.matmul(
                            out=psw[z : z + EMB, cc0 : cc0 + wdt],
                            lhsT=vt[:, 64 * (k % 4) + 8 * g :
                                    64 * (k % 4) + 8 * g + EMB],
                            rhs=ohsc[:, off - o0 : off - o0 + wdt],
                            start=False, stop=(ci == nch - 1),
                            skip_group_check=True)
                    if last:
                        nc.vector.tensor_reduce(
                            vacc[:, w : w + 1], psw[:, :nw],
                            axis=mybir.AxisListType.X, op=ALU.add)
                    else:
                        d8o = wp.tile([P, CAP_N], f32, tag="d8o")
                        nc.scalar.activation(d8o[:, :nw], psw[:, :nw],
                                             AF.Copy)
                        nc.sync.dma_start(
                            out=delta_dram[:, w * CAP_N : w * CAP_N + nw],
                            in_=d8o[:, :nw])

                for w in range(NW + 1):
                    if w < NW:
                        compute_stage(w)
                    if w >= 1:
                        scatter_stage(w - 1)
                if not last:
                    for w in range(NW):
                        n0 = int(pp["win_n0"][w])
                        n1 = int(pp["win_n1"][w])
                        nw = n1 - n0
                        d8 = wp.tile([P, CAP_N], f32, tag="d8")
                        nc.sync.dma_start(
                            out=d8[:, :nw],
                            in_=delta_dram[:, w * CAP_N : w * CAP_N + nw])
                        psr = pp_t.tile([P, 512], f32, tag="pt")
                        nc.tensor.matmul(out=psr[:, :nw], lhsT=lhs_rep[:],
                                         rhs=d8[:, :nw], start=True,
                                         stop=True)
                        nc.vector.tensor_add(table[:, n0:n1],
                                             table[:, n0:n1], psr[:, :nw])

            # ------- phase C: mean + MLP
            tred = cp.tile([P, 2], f32, tag="tred")
            nc.vector.tensor_reduce(tred[0:EMB, 0:1], table[0:EMB, :],
                                    axis=mybir.AxisListType.X, op=ALU.add)
            nc.vector.tensor_reduce(tred[:, 1:2], vacc[:, :NW],
                                    axis=mybir.AxisListType.X, op=ALU.add)
            # fold the three staggered row-sets + table sum on PE, then
            # scale by 1/N on the way out of PSUM
            psv = pp_t.tile([P, 512], f32, tag="pt")
            nc.tensor.matmul(out=psv[:EMB, 0:1], lhsT=lhs_fold[:],
                             rhs=tred[:, 1:2], start=True, stop=False)
            nc.tensor.matmul(out=psv[:EMB, 0:1], lhsT=lhs_id8[:],
                             rhs=tred[:, 0:1], start=False, stop=True)
            vec = cp.tile([EMB, 1], f32, tag="vec")
            nc.scalar.activation(vec[:], psv[:EMB, 0:1], AF.Identity,
                                 scale=1.0 / N)
            psm = pp_t.tile([P, 512], f32, tag="pt")
            nc.tensor.matmul(out=psm[:H1, 0:1], lhsT=w1[:], rhs=vec[:],
                             start=True, stop=True)
            h1t = cp.tile([H1, 1], f32, tag="h1")
            nc.scalar.activation(h1t[:], psm[:H1, 0:1], AF.Relu, bias=b1[:])
            psm2 = pp_t.tile([P, 512], f32, tag="pt")
            nc.tensor.matmul(out=psm2[:H2, 0:1], lhsT=w2[:], rhs=h1t[:],
                             start=True, stop=True)
            h2t = cp.tile([H2, 1], f32, tag="h2")
            nc.scalar.activation(h2t[:], psm2[:H2, 0:1], AF.Relu, bias=b2[:])
            psm3 = pp_t.tile([P, 512], f32, tag="pt")
            nc.tensor.matmul(out=psm3[:1, 0:1], lhsT=w3[:], rhs=h2t[:],
                             start=True, stop=True)
            ot = cp.tile([1, 1], f32, tag="ot")
            nc.scalar.activation(ot[:], psm3[:1, 0:1], AF.Identity,
                                 bias=b3[:])
            nc.sync.dma_start(out=out_p[:], in_=ot[:])

    nc.compile()
    return nc


def _in_maps(pp, cn, sites, bonds, mlp):
    import ml_dtypes
    bf16 = ml_dtypes.bfloat16
    shared = {
        "idx12w": pp["idx12w"],
        "oneh": pp["oneh"].astype(bf16),
        "oh_exp": pp["oh_exp"].astype(bf16),
    }
    for k, v in cn.items():
        if (k == "ident64" or k == "lhsT_wbond" or k.startswith("lhsT_bnd_")
                or k.startswith("lhsT_s2_")):
            shared[k] = v.astype(bf16)
        else:
            shared[k] = v.astype(F32)
    shared.update(mlp)
    in_maps = []
    esrc = pp["esrc"]
    inv = np.full(pp["NV"], N, np.int64)
    inv[pp["newid"]] = np.arange(N)
    for b in range(B):
        bsorted = bonds[b, :, 0][pp["order"]]
        bg = np.where(pp["pad"], 0.0, bsorted[np.clip(esrc, 0, None)])
        m = dict(shared)
        sp = np.concatenate([sites[b, :, 0], [0.0]]).astype(F32)
        m["sites"] = np.ascontiguousarray(sp[inv][None, :], F32)
        m["bonds_g"] = bg.astype(F32)
        in_maps.append(m)
    return in_maps


def kernel(sites, bonds, idx1, idx2, W_site, b_site, W_bond, b_bond,
           W_sig, b_sig, W_sm, b_sm, W1, b1, W2, b2, W3, b3):
    sites = np.asarray(sites, F32)
    bonds = np.asarray(bonds, F32)
    pp = _prep(np.asarray(idx1), np.asarray(idx2))
    cn = _consts(pp, np.asarray(W_site, F32), np.asarray(b_site, F32),
                 np.asarray(W_bond, F32), np.asarray(b_bond, F32),
                 np.asarray(W_sig, F32), np.asarray(b_sig, F32),
                 np.asarray(W_sm, F32), np.asarray(b_sm, F32))
    mlp = {
        "W1": np.asarray(W1, F32), "b1": np.asarray(b1, F32)[:, None],
        "W2": np.asarray(W2, F32), "b2": np.asarray(b2, F32)[:, None],
        "W3": np.asarray(W3, F32), "b3": np.asarray(b3, F32)[:, None],
    }
    nc = _build_bass(pp)
    in_maps = _in_maps(pp, cn, sites, bonds, mlp)
    from concourse.bass_utils import run_bass_kernel_spmd
    res = run_bass_kernel_spmd(nc, in_maps, list(range(B)))
    global LAST_RESULT
    LAST_RESULT = res
    out = np.stack([np.asarray(res.results[b]["out"]).reshape(1)
                    for b in range(B)], axis=0)
    return out.astype(F32)


LAST_RESULT = None

